# revision 8
# baseline (speedup 1.0000x reference)
"""CAGroup3DHead kernel for 8 Trainium2 NeuronCores.

Strategy (data-parallel over voxels, per the sharding hint):
  - Host: integer index work (sorted-key neighbor lookup identical to the
    reference), weight fusion (BN folded into weights, ELU+1 bias shifts),
    and sharding marshaling (transpose to channel-major, bf16 cast,
    per-core slices).  The 3x3x3 sparse conv collapses to a gather: the
    (0,0,0) tap always hits, so conv_in = feats[rep]; the rare other-tap
    hits are folded into conv_in via W_k @ W_13^{-1} so the device conv is
    one dense matmul.
  - Semantic gate fast path: cls and reg_pc are gated by
    sigmoid(sem) > 0.15, i.e. sem_logit + sem_b > logit(0.15).  The host
    checks the rigorous bound max_i ||feats_i|| * max_c ||sem_w_c|| +
    sem_b_c < logit(0.15); when it holds (it does for the detection-prior
    bias -4.595 used here), every mask entry is exactly zero, so cls and
    reg_pc are exactly zero and the device skips them.  If the bound ever
    fails, a full device path (mask + masked heads) is built instead.
  - Device fast path (identical SPMD program on 8 cores): inputs are
    SBUF-resident (a few large DMAs); work proceeds in 1024-column
    supertiles.  Three 128x128 bf16 matmul chains (mlp1, conv, mlp2) with
    ELU+1 done as one scalar Exp + one relu + one min (exact); the three
    skinny heads (sem 18, voff 3, cen 1) share one [65,T] PSUM tile via
    32-row tile_position strips and drain with a single scalar op.
    Outputs are bf16 and stream out per-supertile; the host re-transposes
    and fills the zero sections during unsharding.
"""

import numpy as np
import ml_dtypes

import concourse.bass as bass
import concourse.bacc as bacc
import concourse.tile as tile
from concourse import mybir
from concourse.bass_utils import run_bass_kernel_spmd

BF16 = ml_dtypes.bfloat16

N_VOX = 100000
C = 128
N_CLS = 18
N_REG = 6
VS = 0.04
THR = 0.15
HASH_D = 260
N_CORES = 8
PER_CORE = N_VOX // N_CORES          # 12500
ST = 1024                            # supertile columns
N_ST = 13                            # 12 full + tail of 212
LOGIT_THR = float(np.log(THR / (1.0 - THR)))   # -1.734601..

FOH = 127                            # full-path head rows [cls|cen|regJ]
OUT_ROWS = 151

F32 = mybir.dt.float32
BF = mybir.dt.bfloat16
AOp = mybir.AluOpType
Act = mybir.ActivationFunctionType


def _build_fast():
    """Fast path: masks provably all-zero; compute sem/voff/voted/cen only."""
    nc = bacc.Bacc(trn_type="TRN2")

    xT_d = nc.dram_tensor("xT", [C, PER_CORE], BF, kind="ExternalInput")
    gT_d = nc.dram_tensor("gT", [C, PER_CORE], BF, kind="ExternalInput")
    cvs_d = nc.dram_tensor("cvs", [3, PER_CORE], BF, kind="ExternalInput")
    # bf16 weights: w1 0:128, wc 128:256, w2 256:384, sem 384:402,
    # w3 402:405, cen 405:406
    wb_d = nc.dram_tensor("wb", [C, 406], BF, kind="ExternalInput")
    # f32 per-partition scalars [128, 11]: col0 b1, col1 bc, col2 b2,
    # col3 bias65 (rows 0:18 semb, 32:35 c3, 64 cenb),
    # col4 mn (rows 32:35), col5 mx (rows 32:35), col6-8 b1/bc/b2 + 1
    sc_d = nc.dram_tensor("sc", [C, 11], F32, kind="ExternalInput")
    outH_d = nc.dram_tensor("outH", [65, PER_CORE], BF, kind="ExternalOutput")
    outW_d = nc.dram_tensor("outW", [3, PER_CORE], BF, kind="ExternalOutput")

    chunks = [(0, 3072), (3072, 6144), (6144, 9216), (9216, PER_CORE)]

    with tile.TileContext(nc) as tc:
        with (
            tc.tile_pool(name="res", bufs=1) as res,
            tc.tile_pool(name="epool", bufs=4) as epool,
            tc.tile_pool(name="rpool", bufs=4) as rpool,
            tc.tile_pool(name="fpool", bufs=4) as fpool,
            tc.tile_pool(name="hpool", bufs=3) as hpool,
            tc.tile_pool(name="pbig", bufs=3, space=bass.MemorySpace.PSUM) as pbig,
            tc.tile_pool(name="phead", bufs=1, space=bass.MemorySpace.PSUM) as ph,
        ):
            wb = res.tile([C, 406], BF)
            sc = res.tile([C, 11], F32)
            nc.sync.dma_start(wb[:], wb_d[:])
            nc.sync.dma_start(sc[:], sc_d[:])
            w1 = wb[:, 0:128]
            wc = wb[:, 128:256]
            w2 = wb[:, 256:384]
            semw = wb[:, 384:402]
            w3 = wb[:, 402:405]
            cenw = wb[:, 405:406]
            b1 = sc[:, 0:1]
            bc = sc[:, 1:2]
            b2 = sc[:, 2:3]
            bias65 = sc[0:65, 3:4]
            mn3 = sc[32:35, 4:5]
            mx3 = sc[32:35, 5:6]
            b1p1 = sc[:, 6:7]
            bcp1 = sc[:, 7:8]
            b2p1 = sc[:, 8:9]

            xT = res.tile([C, PER_CORE], BF)
            gT = res.tile([C, PER_CORE], BF)
            cvsr = res.tile([35, PER_CORE], BF)
            for a, b in chunks:
                nc.sync.dma_start(xT[:, a:b], xT_d[:, a:b])
                nc.sync.dma_start(gT[:, a:b], gT_d[:, a:b])
                nc.sync.dma_start(cvsr[32:35, a:b], cvs_d[:, a:b])

            def mm_blocks(p, w, src_sb, off, L, rows=None, tp=None):
                for a in range(0, L, 512):
                    e = min(a + 512, L)
                    dst = p[:, a:e] if rows is None else p[rows[0]:rows[1], a:e]
                    if tp is None:
                        nc.tensor.matmul(dst, w, src_sb[:, off + a:off + e],
                                         start=True, stop=True)
                    else:
                        nc.tensor.matmul(dst, w, src_sb[:, off + a:off + e],
                                         start=True, stop=True,
                                         tile_position=tp)

            def elu_tail(p, bias, biasp1, L):
                # f = min(max(y+b+1, 1), exp(y+b)) == ELU(y+b) + 1, exact
                et = epool.tile([C, ST], BF, tag="e")
                nc.scalar.activation(et[:, :L], p[:, :L], Act.Exp, bias=bias)
                rt = rpool.tile([C, ST], BF, tag="r")
                nc.vector.tensor_scalar(rt[:, :L], p[:, :L], biasp1, 1.0,
                                        AOp.add, AOp.max)
                ft = fpool.tile([C, ST], BF, tag="f")
                nc.vector.tensor_tensor(ft[:, :L], rt[:, :L], et[:, :L],
                                        AOp.min)
                return ft

            for k in range(N_ST):
                cs = k * ST
                L = min(ST, PER_CORE - cs)

                # mlp1 + conv matmuls, then sem (xT-only) early for overlap
                p1 = pbig.tile([C, ST], F32, tag="p")
                mm_blocks(p1, w1, xT, cs, L)
                pc = pbig.tile([C, ST], F32, tag="p")
                mm_blocks(pc, wc, gT, cs, L)
                phd = ph.tile([65, ST], F32, tag="ph")
                mm_blocks(phd, semw, xT, cs, L, rows=(0, 18), tp=(0, 0))

                f1 = elu_tail(p1, b1, b1p1, L)
                fo = elu_tail(pc, bc, bcp1, L)

                p2 = pbig.tile([C, ST], F32, tag="p")
                mm_blocks(p2, w2, f1, 0, L)
                mm_blocks(phd, cenw, fo, 0, L, rows=(64, 65), tp=(0, 64))
                f2 = elu_tail(p2, b2, b2p1, L)
                mm_blocks(phd, w3, f2, 0, L, rows=(32, 35), tp=(0, 32))

                sv = hpool.tile([65, ST], BF, tag="sv")
                nc.scalar.activation(sv[:, :L], phd[:, :L], Act.Identity,
                                     bias=bias65)
                nc.sync.dma_start(outH_d[:, cs:cs + L], sv[:, :L])

                # ---- voted = clip(voff + coords*VS, mn, mx) on gpsimd ----
                vp = hpool.tile([35, ST], BF, tag="vp")
                nc.gpsimd.tensor_tensor(vp[32:35, :L], sv[32:35, :L],
                                        cvsr[32:35, cs:cs + L], AOp.add)
                vt = hpool.tile([35, ST], BF, tag="vt")
                nc.gpsimd.tensor_scalar(vt[32:35, :L], vp[32:35, :L],
                                        mn3, mx3, AOp.max, AOp.min)
                nc.sync.dma_start(outW_d[:, cs:cs + L], vt[32:35, :L])

    nc.finalize()
    return nc


def _build_full():
    """Fallback: full mask + masked-heads path (used if the zero-mask
    bound fails).  j-major reg layout, one is_gt + one STT for heads."""
    nc = bacc.Bacc(trn_type="TRN2")

    xT_d = nc.dram_tensor("xT", [C, PER_CORE], BF, kind="ExternalInput")
    gT_d = nc.dram_tensor("gT", [C, PER_CORE], BF, kind="ExternalInput")
    cvs_d = nc.dram_tensor("cvs", [3, PER_CORE], BF, kind="ExternalInput")
    # w1 0:128, wc 128:256, w2 256:384, semJF 384:511, fohF 511:638,
    # w3 638:641, sem 641:659, cen 659:660
    wb_d = nc.dram_tensor("wb", [C, 660], BF, kind="ExternalInput")
    # col0 b2, col1 bias65, col2 mn, col3 mx, col4 thrF, col5 biasF
    sc_d = nc.dram_tensor("sc", [C, 8], F32, kind="ExternalInput")
    outH_d = nc.dram_tensor("outH", [65, PER_CORE], BF, kind="ExternalOutput")
    outW_d = nc.dram_tensor("outW", [3, PER_CORE], BF, kind="ExternalOutput")
    outB_d = nc.dram_tensor("outB", [FOH, PER_CORE], BF, kind="ExternalOutput")

    chunks = [(0, 3072), (3072, 6144), (6144, 9216), (9216, PER_CORE)]

    with tile.TileContext(nc) as tc:
        with (
            tc.tile_pool(name="res", bufs=1) as res,
            tc.tile_pool(name="epool", bufs=4) as epool,
            tc.tile_pool(name="rpool", bufs=4) as rpool,
            tc.tile_pool(name="fpool", bufs=4) as fpool,
            tc.tile_pool(name="hpool", bufs=3) as hpool,
            tc.tile_pool(name="mpool", bufs=2) as mpool,
            tc.tile_pool(name="pbig", bufs=2, space=bass.MemorySpace.PSUM) as pbig,
            tc.tile_pool(name="phead", bufs=1, space=bass.MemorySpace.PSUM) as ph,
            tc.tile_pool(name="pfoh", bufs=1, space=bass.MemorySpace.PSUM) as pf,
        ):
            wb = res.tile([C, 660], BF)
            sc = res.tile([C, 11], F32)
            nc.sync.dma_start(wb[:], wb_d[:])
            nc.sync.dma_start(sc[:], sc_d[:])
            w1 = wb[:, 0:128]
            wc = wb[:, 128:256]
            w2 = wb[:, 256:384]
            semJF = wb[:, 384:511]
            fohF = wb[:, 511:638]
            w3 = wb[:, 638:641]
            semw = wb[:, 641:659]
            cenw = wb[:, 659:660]
            b1 = sc[:, 0:1]
            bc = sc[:, 1:2]
            b2 = sc[:, 2:3]
            bias65 = sc[0:65, 3:4]
            mn3 = sc[32:35, 4:5]
            mx3 = sc[32:35, 5:6]
            b1p1 = sc[:, 6:7]
            bcp1 = sc[:, 7:8]
            b2p1 = sc[:, 8:9]
            thrF = sc[0:FOH, 9:10]
            biasF = sc[0:FOH, 10:11]

            xT = res.tile([C, PER_CORE], BF)
            gT = res.tile([C, PER_CORE], BF)
            cvsr = res.tile([35, PER_CORE], BF)
            for a, b in chunks:
                nc.sync.dma_start(xT[:, a:b], xT_d[:, a:b])
                nc.sync.dma_start(gT[:, a:b], gT_d[:, a:b])
                nc.sync.dma_start(cvsr[32:35, a:b], cvs_d[:, a:b])

            for k in range(N_ST):
                cs = k * ST
                L = min(ST, PER_CORE - cs)

                def elu_layer(w, src_sb, off, bias):
                    p = pbig.tile([C, ST], F32, tag="p")
                    for a in range(0, L, 512):
                        e = min(a + 512, L)
                        nc.tensor.matmul(p[:, a:e], w,
                                         src_sb[:, off + a:off + e],
                                         start=True, stop=True)
                    et = epool.tile([C, ST], BF, tag="e")
                    nc.scalar.activation(et[:, :L], p[:, :L], Act.Exp,
                                         bias=bias)
                    rt = rpool.tile([C, ST], BF, tag="r")
                    nc.vector.tensor_scalar(rt[:, :L], p[:, :L], bias,
                                            0.0, AOp.add, AOp.max)
                    ft = fpool.tile([C, ST], BF, tag="f")
                    nc.vector.scalar_tensor_tensor(
                        ft[:, :L], rt[:, :L], 1.0, et[:, :L],
                        AOp.add, AOp.min)
                    return ft

                f1 = elu_layer(w1, xT, cs, b1)
                fo = elu_layer(wc, gT, cs, bc)
                f2 = elu_layer(w2, f1, 0, b2)

                phd = ph.tile([65, ST], F32, tag="ph")
                for a in range(0, L, 512):
                    e = min(a + 512, L)
                    nc.tensor.matmul(phd[0:18, a:e], semw,
                                     xT[:, cs + a:cs + e],
                                     start=True, stop=True,
                                     tile_position=(0, 0))
                    nc.tensor.matmul(phd[32:35, a:e], w3, f2[:, a:e],
                                     start=True, stop=True,
                                     tile_position=(0, 32))
                    nc.tensor.matmul(phd[64:65, a:e], cenw, fo[:, a:e],
                                     start=True, stop=True,
                                     tile_position=(0, 64))
                sv = hpool.tile([65, ST], BF, tag="sv")
                nc.scalar.activation(sv[:, :L], phd[:, :L], Act.Identity,
                                     bias=bias65)
                nc.sync.dma_start(outH_d[:, cs:cs + L], sv[:, :L])

                vp = hpool.tile([35, ST], BF, tag="vp")
                nc.gpsimd.tensor_tensor(vp[32:35, :L], sv[32:35, :L],
                                        cvsr[32:35, cs:cs + L], AOp.add)
                vt = hpool.tile([35, ST], BF, tag="vt")
                nc.vector.tensor_scalar(vt[32:35, :L], vp[32:35, :L],
                                        mn3, mx3, AOp.max, AOp.min)
                nc.sync.dma_start(outW_d[:, cs:cs + L], vt[32:35, :L])

                # masked heads: (foh + biasF) * (semJ > thrF)
                psem = ph.tile([FOH, ST], F32, tag="ph")
                for a in range(0, L, 512):
                    e = min(a + 512, L)
                    nc.tensor.matmul(psem[:, a:e], semJF,
                                     xT[:, cs + a:cs + e],
                                     start=True, stop=True)
                maskF = mpool.tile([FOH, ST], BF, tag="maskF")
                nc.vector.tensor_scalar(maskF[:, :L], psem[:, :L], thrF,
                                        None, AOp.is_gt)
                pfo = pf.tile([FOH, ST], F32, tag="pfoh")
                for a in range(0, L, 512):
                    e = min(a + 512, L)
                    nc.tensor.matmul(pfo[:, a:e], fohF, fo[:, a:e],
                                     start=True, stop=True)
                hb = mpool.tile([FOH, ST], BF, tag="hb")
                nc.vector.scalar_tensor_tensor(
                    hb[:, :L], pfo[:, :L], biasF, maskF[:, :L],
                    AOp.add, AOp.mult)
                nc.sync.dma_start(outB_d[:, cs:cs + L], hb[:, :L])

    nc.finalize()
    return nc


def _host_prep(feats, coords_xyz, batch_idx,
               off_w1, off_g1, off_b1, off_w2, off_g2, off_b2, off_w3,
               fo_w, fo_g, fo_b, sem_w, sem_b, cen_w, cls_w, cls_b, reg_w,
               scales):
    f64 = np.float64
    N = feats.shape[0]

    # ---- zero-mask bound: |x.w_c| <= max||x|| * ||w_c|| (Cauchy-Schwarz)
    feats64 = feats.astype(f64)
    max_row = np.sqrt((feats64 * feats64).sum(1).max())
    colnrm = np.sqrt((sem_w.astype(f64) ** 2).sum(0))
    zero_mask = bool(
        np.all(max_row * colnrm + sem_b.astype(f64) < LOGIT_THR - 1e-3))

    # ---- neighbor lookup (identical to reference's sorted-key search) ----
    c1 = coords_xyz.astype(np.int64) + 1
    key = ((batch_idx.astype(np.int64) * HASH_D + c1[:, 0]) * HASH_D
           + c1[:, 1]) * HASH_D + c1[:, 2]
    order = np.argsort(key, kind="stable")
    skey = key[order]
    pos = np.searchsorted(skey, key)
    rep = order[pos]                      # first voxel with same key

    # ---- fused weights (BN folded; ELU+1 handled via bias shifts) ----
    W1 = off_w1.astype(f64) * off_g1.astype(f64)[None, :]
    b1 = off_b1.astype(f64)
    W2 = off_w2.astype(f64) * off_g2.astype(f64)[None, :]
    b2 = off_b2.astype(f64) - W2.sum(0)
    W3 = off_w3.astype(f64)
    c3 = -W3.sum(0)
    Wc = fo_w[13].astype(f64) * fo_g.astype(f64)[None, :]
    bc = fo_b.astype(f64)

    # ---- conv input: gather + fold rare non-center taps via Wc13^-1 ----
    G = feats64[rep]
    Winv = np.linalg.inv(fo_w[13].astype(f64))
    k = 0
    for dx in (-1, 0, 1):
        for dy in (-1, 0, 1):
            for dz in (-1, 0, 1):
                if (dx, dy, dz) != (0, 0, 0):
                    nk = key + (dx * HASH_D + dy) * HASH_D + dz
                    p = np.clip(np.searchsorted(skey, nk), 0, N - 1)
                    hit = skey[p] == nk
                    if hit.any():
                        dst = np.nonzero(hit)[0]
                        src = order[p[hit]]
                        A = fo_w[k].astype(f64) @ Winv
                        np.add.at(G, dst, feats64[src] @ A)
                k += 1

    # ---- shared scalar columns ----
    mx = (coords_xyz.max(0) + 1).astype(f64) * VS
    mn = (coords_xyz.min(0) - 1).astype(f64) * VS
    bias65 = np.zeros(65, f64)
    bias65[0:N_CLS] = sem_b.astype(f64)
    bias65[32:35] = c3
    bias65[64] = -cen_w.astype(f64).sum(0)[0]

    sc = np.zeros((C, 11), np.float32)
    sc[:, 0] = b1
    sc[:, 1] = bc
    sc[:, 2] = b2
    sc[0:65, 3] = bias65
    sc[32:35, 4] = mn
    sc[32:35, 5] = mx
    sc[:, 6] = b1 + 1.0
    sc[:, 7] = bc + 1.0
    sc[:, 8] = b2 + 1.0

    nwb = 406 if zero_mask else 660
    wb = np.zeros((C, nwb), BF16)
    wb[:, 0:128] = W1.astype(BF16)
    wb[:, 128:256] = Wc.astype(BF16)
    wb[:, 256:384] = W2.astype(BF16)
    if zero_mask:
        wb[:, 384:402] = sem_w.astype(f64).astype(BF16)
        wb[:, 402:405] = W3.astype(BF16)
        wb[:, 405:406] = cen_w.astype(f64).astype(BF16)
    else:
        # j-major layout: rows [cls 0:18 | cen 18 | regJ 19:127]
        sc64 = scales.astype(f64)
        semJF = np.zeros((C, FOH), f64)
        fohF = np.zeros((C, FOH), f64)
        thrF = np.zeros(FOH, np.float32)
        biasF = np.zeros(FOH, np.float32)
        semJF[:, 0:N_CLS] = sem_w.astype(f64)
        fohF[:, 0:N_CLS] = cls_w.astype(f64)
        thrF[0:N_CLS] = LOGIT_THR - sem_b.astype(f64)
        biasF[0:N_CLS] = cls_b.astype(f64) - cls_w.astype(f64).sum(0)
        fohF[:, N_CLS] = cen_w.astype(f64)[:, 0]
        thrF[N_CLS] = -1e30
        biasF[N_CLS] = -cen_w.astype(f64).sum(0)[0]
        for j in range(N_REG):
            for cc in range(N_CLS):
                r = 19 + j * N_CLS + cc
                semJF[:, r] = sem_w.astype(f64)[:, cc]
                fohF[:, r] = reg_w.astype(f64)[:, j] * sc64[cc]
                thrF[r] = LOGIT_THR - sem_b.astype(f64)[cc]
                biasF[r] = -reg_w.astype(f64)[:, j].sum() * sc64[cc]
        wb[:, 384:511] = semJF.astype(BF16)
        wb[:, 511:638] = fohF.astype(BF16)
        wb[:, 638:641] = W3.astype(BF16)
        wb[:, 641:659] = sem_w.astype(f64).astype(BF16)
        wb[:, 659:660] = cen_w.astype(f64).astype(BF16)
        sc[0:FOH, 9] = thrF
        sc[0:FOH, 10] = biasF

    # ---- transposed, channel-major activations ----
    fT = np.ascontiguousarray(feats.T)
    gTf = np.ascontiguousarray(G.astype(np.float32).T)
    cT = coords_xyz.T.astype(np.float32) * VS

    wts = {"wb": wb, "sc": sc}
    in_maps = []
    for c in range(N_CORES):
        s, e = c * PER_CORE, (c + 1) * PER_CORE
        m = dict(wts)
        m["xT"] = np.ascontiguousarray(fT[:, s:e]).astype(BF16)
        m["gT"] = np.ascontiguousarray(gTf[:, s:e]).astype(BF16)
        m["cvs"] = np.ascontiguousarray(cT[:, s:e]).astype(BF16)
        in_maps.append(m)
    return in_maps, zero_mask


_CACHED = {}

# full path: regJ row 19 + j*18 + c  ->  output col 43 + c*6 + j
_REG_COLS = np.empty(FOH - 19, np.int64)
for _j in range(N_REG):
    for _c in range(N_CLS):
        _REG_COLS[_j * N_CLS + _c] = 43 + _c * N_REG + _j


def _assemble(res, zero_mask, n):
    """Map device outputs to reference layout [n, 151] f32."""
    o = np.zeros((n, OUT_ROWS), np.float32)
    outH = res["outH"][:, :n].astype(np.float32)
    o[:, 0:18] = outH[0:18].T
    o[:, 18:21] = outH[32:35].T
    o[:, 21:24] = res["outW"][:, :n].astype(np.float32).T
    o[:, 24] = outH[64]
    if not zero_mask:
        outB = res["outB"][:, :n].astype(np.float32)
        o[:, 25:43] = outB[0:N_CLS].T
        o[:, _REG_COLS] = outB[19:FOH].T
    return o


def kernel(**inputs):
    inputs = {k: np.asarray(v) for k, v in inputs.items()}
    in_maps, zero_mask = _host_prep(**inputs)
    ckey = "fast" if zero_mask else "full"
    if ckey not in _CACHED:
        _CACHED[ckey] = _build_fast() if zero_mask else _build_full()
    nc = _CACHED[ckey]
    res = run_bass_kernel_spmd(nc, in_maps, core_ids=list(range(N_CORES)))
    out = np.empty((N_VOX, OUT_ROWS), np.float32)
    for c in range(N_CORES):
        out[c * PER_CORE:(c + 1) * PER_CORE] = _assemble(
            res.results[c], zero_mask, PER_CORE)
    return out


# revision 9
# speedup vs baseline: 2.0649x; 2.0649x over previous
"""CAGroup3DHead kernel for 8 Trainium2 NeuronCores.

Strategy (data-parallel over voxels, per the sharding hint):
  - Host: integer index work (sorted-key neighbor lookup identical to the
    reference), weight fusion (BN folded into weights, ELU+1 bias shifts),
    and sharding marshaling (transpose to channel-major, bf16 cast,
    per-core slices).  The 3x3x3 sparse conv collapses to a gather: the
    (0,0,0) tap always hits, so conv_in = feats[rep]; the rare other-tap
    hits are folded into conv_in via W_k @ W_13^{-1} so the device conv is
    one dense matmul.
  - Semantic gate fast path: cls and reg_pc are gated by
    sigmoid(sem) > 0.15, i.e. sem_logit + sem_b > logit(0.15).  The host
    checks the rigorous bound max_i ||feats_i|| * max_c ||sem_w_c|| +
    sem_b_c < logit(0.15); when it holds (it does for the detection-prior
    bias -4.595 used here), every mask entry is exactly zero, so cls and
    reg_pc are exactly zero and the device skips them.  If the bound ever
    fails, a full device path (mask + masked heads) is built instead.
  - Device fast path (identical SPMD program on 8 cores): inputs are
    SBUF-resident (a few large DMAs); work proceeds in 1024-column
    supertiles.  Three 128x128 bf16 matmul chains (mlp1, conv, mlp2) with
    ELU+1 done as one scalar Exp + one relu + one min (exact); the three
    skinny heads (sem 18, voff 3, cen 1) share one [65,T] PSUM tile via
    32-row tile_position strips and drain with a single scalar op.
    Outputs are bf16 and stream out per-supertile; the host re-transposes
    and fills the zero sections during unsharding.
"""

import numpy as np
import ml_dtypes

import concourse.bass as bass
import concourse.bacc as bacc
import concourse.tile as tile
from concourse import mybir
from concourse.bass_utils import run_bass_kernel_spmd

BF16 = ml_dtypes.bfloat16

N_VOX = 100000
C = 128
N_CLS = 18
N_REG = 6
VS = 0.04
THR = 0.15
HASH_D = 260
N_CORES = 8
PER_CORE = N_VOX // N_CORES          # 12500
ST = 1024                            # supertile columns
N_ST = 13                            # 12 full + tail of 212
LOGIT_THR = float(np.log(THR / (1.0 - THR)))   # -1.734601..

FOH = 127                            # full-path head rows [cls|cen|regJ]
OUT_ROWS = 151

F32 = mybir.dt.float32
BF = mybir.dt.bfloat16
AOp = mybir.AluOpType
Act = mybir.ActivationFunctionType


def _build_fast():
    """Fast path: masks provably all-zero; compute sem/voff/voted/cen only."""
    nc = bacc.Bacc(trn_type="TRN2")

    xT_d = nc.dram_tensor("xT", [C, PER_CORE], BF, kind="ExternalInput")
    gT_d = nc.dram_tensor("gT", [C, PER_CORE], BF, kind="ExternalInput")
    cvs_d = nc.dram_tensor("cvs", [3, PER_CORE], BF, kind="ExternalInput")
    # bf16 weights: w1 0:128, wc 128:256, w2 256:384, sem 384:402,
    # w3 402:405, cen 405:406
    wb_d = nc.dram_tensor("wb", [C, 406], BF, kind="ExternalInput")
    # f32 per-partition scalars [128, 11]: col0 b1, col1 bc, col2 b2,
    # col3 bias65 (rows 0:18 semb, 32:35 c3, 64 cenb),
    # col4 mn (rows 32:35), col5 mx (rows 32:35), col6-8 b1/bc/b2 + 1
    sc_d = nc.dram_tensor("sc", [C, 11], F32, kind="ExternalInput")
    outH_d = nc.dram_tensor("outH", [65, PER_CORE], BF, kind="ExternalOutput")
    outW_d = nc.dram_tensor("outW", [3, PER_CORE], BF, kind="ExternalOutput")

    chunks = [(0, 3072), (3072, 6144), (6144, 9216), (9216, PER_CORE)]

    with tile.TileContext(nc) as tc:
        with (
            tc.tile_pool(name="res", bufs=1) as res,
            tc.tile_pool(name="epool", bufs=4) as epool,
            tc.tile_pool(name="rpool", bufs=4) as rpool,
            tc.tile_pool(name="fpool", bufs=4) as fpool,
            tc.tile_pool(name="hpool", bufs=3) as hpool,
            tc.tile_pool(name="pbig", bufs=3, space=bass.MemorySpace.PSUM) as pbig,
            tc.tile_pool(name="phead", bufs=1, space=bass.MemorySpace.PSUM) as ph,
        ):
            wb = res.tile([C, 406], BF)
            sc = res.tile([C, 11], F32)
            nc.sync.dma_start(wb[:], wb_d[:])
            nc.sync.dma_start(sc[:], sc_d[:])
            w1 = wb[:, 0:128]
            wc = wb[:, 128:256]
            w2 = wb[:, 256:384]
            semw = wb[:, 384:402]
            w3 = wb[:, 402:405]
            cenw = wb[:, 405:406]
            b1 = sc[:, 0:1]
            bc = sc[:, 1:2]
            b2 = sc[:, 2:3]
            bias65 = sc[0:65, 3:4]
            mn3 = sc[32:35, 4:5]
            mx3 = sc[32:35, 5:6]
            b1p1 = sc[:, 6:7]
            bcp1 = sc[:, 7:8]
            b2p1 = sc[:, 8:9]

            xT = res.tile([C, PER_CORE], BF)
            gT = res.tile([C, PER_CORE], BF)
            cvsr = res.tile([35, PER_CORE], BF)
            for a, b in chunks:
                nc.sync.dma_start(xT[:, a:b], xT_d[:, a:b])
                nc.sync.dma_start(gT[:, a:b], gT_d[:, a:b])
                nc.sync.dma_start(cvsr[32:35, a:b], cvs_d[:, a:b])

            def mm_blocks(p, w, src_sb, off, L, rows=None, tp=None):
                for a in range(0, L, 512):
                    e = min(a + 512, L)
                    dst = p[:, a:e] if rows is None else p[rows[0]:rows[1], a:e]
                    if tp is None:
                        nc.tensor.matmul(dst, w, src_sb[:, off + a:off + e],
                                         start=True, stop=True)
                    else:
                        nc.tensor.matmul(dst, w, src_sb[:, off + a:off + e],
                                         start=True, stop=True,
                                         tile_position=tp)

            def elu_tail(p, bias, biasp1, L):
                # f = min(max(y+b+1, 1), exp(y+b)) == ELU(y+b) + 1, exact
                et = epool.tile([C, ST], BF, tag="e")
                nc.scalar.activation(et[:, :L], p[:, :L], Act.Exp, bias=bias)
                rt = rpool.tile([C, ST], BF, tag="r")
                nc.vector.tensor_scalar(rt[:, :L], p[:, :L], biasp1, 1.0,
                                        AOp.add, AOp.max)
                ft = fpool.tile([C, ST], BF, tag="f")
                nc.vector.tensor_tensor(ft[:, :L], rt[:, :L], et[:, :L],
                                        AOp.min)
                return ft

            for k in range(N_ST):
                cs = k * ST
                L = min(ST, PER_CORE - cs)

                # mlp1 + conv matmuls, then sem (xT-only) early for overlap
                p1 = pbig.tile([C, ST], F32, tag="p")
                mm_blocks(p1, w1, xT, cs, L)
                pc = pbig.tile([C, ST], F32, tag="p")
                mm_blocks(pc, wc, gT, cs, L)
                phd = ph.tile([65, ST], F32, tag="ph")
                mm_blocks(phd, semw, xT, cs, L, rows=(0, 18), tp=(0, 0))

                f1 = elu_tail(p1, b1, b1p1, L)
                fo = elu_tail(pc, bc, bcp1, L)

                p2 = pbig.tile([C, ST], F32, tag="p")
                mm_blocks(p2, w2, f1, 0, L)
                mm_blocks(phd, cenw, fo, 0, L, rows=(64, 65), tp=(0, 64))
                f2 = elu_tail(p2, b2, b2p1, L)
                mm_blocks(phd, w3, f2, 0, L, rows=(32, 35), tp=(0, 32))

                sv = hpool.tile([65, ST], BF, tag="sv")
                nc.scalar.activation(sv[:, :L], phd[:, :L], Act.Identity,
                                     bias=bias65)
                nc.sync.dma_start(outH_d[:, cs:cs + L], sv[:, :L])

                # ---- voted = clip(voff + coords*VS, mn, mx) on gpsimd ----
                vp = hpool.tile([35, ST], BF, tag="vp")
                nc.gpsimd.tensor_tensor(vp[32:35, :L], sv[32:35, :L],
                                        cvsr[32:35, cs:cs + L], AOp.add)
                vt = hpool.tile([35, ST], BF, tag="vt")
                nc.vector.tensor_scalar(vt[32:35, :L], vp[32:35, :L],
                                        mn3, mx3, AOp.max, AOp.min)
                nc.sync.dma_start(outW_d[:, cs:cs + L], vt[32:35, :L])

    nc.finalize()
    return nc


def _build_full():
    """Fallback: full mask + masked-heads path (used if the zero-mask
    bound fails).  j-major reg layout, one is_gt + one STT for heads."""
    nc = bacc.Bacc(trn_type="TRN2")

    xT_d = nc.dram_tensor("xT", [C, PER_CORE], BF, kind="ExternalInput")
    gT_d = nc.dram_tensor("gT", [C, PER_CORE], BF, kind="ExternalInput")
    cvs_d = nc.dram_tensor("cvs", [3, PER_CORE], BF, kind="ExternalInput")
    # w1 0:128, wc 128:256, w2 256:384, semJF 384:511, fohF 511:638,
    # w3 638:641, sem 641:659, cen 659:660
    wb_d = nc.dram_tensor("wb", [C, 660], BF, kind="ExternalInput")
    # col0 b2, col1 bias65, col2 mn, col3 mx, col4 thrF, col5 biasF
    sc_d = nc.dram_tensor("sc", [C, 8], F32, kind="ExternalInput")
    outH_d = nc.dram_tensor("outH", [65, PER_CORE], BF, kind="ExternalOutput")
    outW_d = nc.dram_tensor("outW", [3, PER_CORE], BF, kind="ExternalOutput")
    outB_d = nc.dram_tensor("outB", [FOH, PER_CORE], BF, kind="ExternalOutput")

    chunks = [(0, 3072), (3072, 6144), (6144, 9216), (9216, PER_CORE)]

    with tile.TileContext(nc) as tc:
        with (
            tc.tile_pool(name="res", bufs=1) as res,
            tc.tile_pool(name="epool", bufs=4) as epool,
            tc.tile_pool(name="rpool", bufs=4) as rpool,
            tc.tile_pool(name="fpool", bufs=4) as fpool,
            tc.tile_pool(name="hpool", bufs=3) as hpool,
            tc.tile_pool(name="mpool", bufs=2) as mpool,
            tc.tile_pool(name="pbig", bufs=2, space=bass.MemorySpace.PSUM) as pbig,
            tc.tile_pool(name="phead", bufs=1, space=bass.MemorySpace.PSUM) as ph,
            tc.tile_pool(name="pfoh", bufs=1, space=bass.MemorySpace.PSUM) as pf,
        ):
            wb = res.tile([C, 660], BF)
            sc = res.tile([C, 11], F32)
            nc.sync.dma_start(wb[:], wb_d[:])
            nc.sync.dma_start(sc[:], sc_d[:])
            w1 = wb[:, 0:128]
            wc = wb[:, 128:256]
            w2 = wb[:, 256:384]
            semJF = wb[:, 384:511]
            fohF = wb[:, 511:638]
            w3 = wb[:, 638:641]
            semw = wb[:, 641:659]
            cenw = wb[:, 659:660]
            b1 = sc[:, 0:1]
            bc = sc[:, 1:2]
            b2 = sc[:, 2:3]
            bias65 = sc[0:65, 3:4]
            mn3 = sc[32:35, 4:5]
            mx3 = sc[32:35, 5:6]
            b1p1 = sc[:, 6:7]
            bcp1 = sc[:, 7:8]
            b2p1 = sc[:, 8:9]
            thrF = sc[0:FOH, 9:10]
            biasF = sc[0:FOH, 10:11]

            xT = res.tile([C, PER_CORE], BF)
            gT = res.tile([C, PER_CORE], BF)
            cvsr = res.tile([35, PER_CORE], BF)
            for a, b in chunks:
                nc.sync.dma_start(xT[:, a:b], xT_d[:, a:b])
                nc.sync.dma_start(gT[:, a:b], gT_d[:, a:b])
                nc.sync.dma_start(cvsr[32:35, a:b], cvs_d[:, a:b])

            for k in range(N_ST):
                cs = k * ST
                L = min(ST, PER_CORE - cs)

                def elu_layer(w, src_sb, off, bias):
                    p = pbig.tile([C, ST], F32, tag="p")
                    for a in range(0, L, 512):
                        e = min(a + 512, L)
                        nc.tensor.matmul(p[:, a:e], w,
                                         src_sb[:, off + a:off + e],
                                         start=True, stop=True)
                    et = epool.tile([C, ST], BF, tag="e")
                    nc.scalar.activation(et[:, :L], p[:, :L], Act.Exp,
                                         bias=bias)
                    rt = rpool.tile([C, ST], BF, tag="r")
                    nc.vector.tensor_scalar(rt[:, :L], p[:, :L], bias,
                                            0.0, AOp.add, AOp.max)
                    ft = fpool.tile([C, ST], BF, tag="f")
                    nc.vector.scalar_tensor_tensor(
                        ft[:, :L], rt[:, :L], 1.0, et[:, :L],
                        AOp.add, AOp.min)
                    return ft

                f1 = elu_layer(w1, xT, cs, b1)
                fo = elu_layer(wc, gT, cs, bc)
                f2 = elu_layer(w2, f1, 0, b2)

                phd = ph.tile([65, ST], F32, tag="ph")
                for a in range(0, L, 512):
                    e = min(a + 512, L)
                    nc.tensor.matmul(phd[0:18, a:e], semw,
                                     xT[:, cs + a:cs + e],
                                     start=True, stop=True,
                                     tile_position=(0, 0))
                    nc.tensor.matmul(phd[32:35, a:e], w3, f2[:, a:e],
                                     start=True, stop=True,
                                     tile_position=(0, 32))
                    nc.tensor.matmul(phd[64:65, a:e], cenw, fo[:, a:e],
                                     start=True, stop=True,
                                     tile_position=(0, 64))
                sv = hpool.tile([65, ST], BF, tag="sv")
                nc.scalar.activation(sv[:, :L], phd[:, :L], Act.Identity,
                                     bias=bias65)
                nc.sync.dma_start(outH_d[:, cs:cs + L], sv[:, :L])

                vp = hpool.tile([35, ST], BF, tag="vp")
                nc.gpsimd.tensor_tensor(vp[32:35, :L], sv[32:35, :L],
                                        cvsr[32:35, cs:cs + L], AOp.add)
                vt = hpool.tile([35, ST], BF, tag="vt")
                nc.vector.tensor_scalar(vt[32:35, :L], vp[32:35, :L],
                                        mn3, mx3, AOp.max, AOp.min)
                nc.sync.dma_start(outW_d[:, cs:cs + L], vt[32:35, :L])

                # masked heads: (foh + biasF) * (semJ > thrF)
                psem = ph.tile([FOH, ST], F32, tag="ph")
                for a in range(0, L, 512):
                    e = min(a + 512, L)
                    nc.tensor.matmul(psem[:, a:e], semJF,
                                     xT[:, cs + a:cs + e],
                                     start=True, stop=True)
                maskF = mpool.tile([FOH, ST], BF, tag="maskF")
                nc.vector.tensor_scalar(maskF[:, :L], psem[:, :L], thrF,
                                        None, AOp.is_gt)
                pfo = pf.tile([FOH, ST], F32, tag="pfoh")
                for a in range(0, L, 512):
                    e = min(a + 512, L)
                    nc.tensor.matmul(pfo[:, a:e], fohF, fo[:, a:e],
                                     start=True, stop=True)
                hb = mpool.tile([FOH, ST], BF, tag="hb")
                nc.vector.scalar_tensor_tensor(
                    hb[:, :L], pfo[:, :L], biasF, maskF[:, :L],
                    AOp.add, AOp.mult)
                nc.sync.dma_start(outB_d[:, cs:cs + L], hb[:, :L])

    nc.finalize()
    return nc


def _host_prep(feats, coords_xyz, batch_idx,
               off_w1, off_g1, off_b1, off_w2, off_g2, off_b2, off_w3,
               fo_w, fo_g, fo_b, sem_w, sem_b, cen_w, cls_w, cls_b, reg_w,
               scales):
    f64 = np.float64
    N = feats.shape[0]

    # ---- zero-mask bound: |x.w_c| <= max||x|| * ||w_c|| (Cauchy-Schwarz)
    feats64 = feats.astype(f64)
    max_row = np.sqrt((feats64 * feats64).sum(1).max())
    colnrm = np.sqrt((sem_w.astype(f64) ** 2).sum(0))
    zero_mask = bool(
        np.all(max_row * colnrm + sem_b.astype(f64) < LOGIT_THR - 1e-3))

    # ---- neighbor lookup (identical to reference's sorted-key search) ----
    c1 = coords_xyz.astype(np.int64) + 1
    key = ((batch_idx.astype(np.int64) * HASH_D + c1[:, 0]) * HASH_D
           + c1[:, 1]) * HASH_D + c1[:, 2]
    order = np.argsort(key, kind="stable")
    skey = key[order]
    pos = np.searchsorted(skey, key)
    rep = order[pos]                      # first voxel with same key

    # ---- fused weights (BN folded; ELU+1 handled via bias shifts) ----
    W1 = off_w1.astype(f64) * off_g1.astype(f64)[None, :]
    b1 = off_b1.astype(f64)
    W2 = off_w2.astype(f64) * off_g2.astype(f64)[None, :]
    b2 = off_b2.astype(f64) - W2.sum(0)
    W3 = off_w3.astype(f64)
    c3 = -W3.sum(0)
    Wc = fo_w[13].astype(f64) * fo_g.astype(f64)[None, :]
    bc = fo_b.astype(f64)

    # ---- conv input: gather + fold rare non-center taps via Wc13^-1 ----
    G = feats64[rep]
    Winv = np.linalg.inv(fo_w[13].astype(f64))
    k = 0
    for dx in (-1, 0, 1):
        for dy in (-1, 0, 1):
            for dz in (-1, 0, 1):
                if (dx, dy, dz) != (0, 0, 0):
                    nk = key + (dx * HASH_D + dy) * HASH_D + dz
                    p = np.clip(np.searchsorted(skey, nk), 0, N - 1)
                    hit = skey[p] == nk
                    if hit.any():
                        dst = np.nonzero(hit)[0]
                        src = order[p[hit]]
                        A = fo_w[k].astype(f64) @ Winv
                        np.add.at(G, dst, feats64[src] @ A)
                k += 1

    # ---- shared scalar columns ----
    mx = (coords_xyz.max(0) + 1).astype(f64) * VS
    mn = (coords_xyz.min(0) - 1).astype(f64) * VS
    bias65 = np.zeros(65, f64)
    bias65[0:N_CLS] = sem_b.astype(f64)
    bias65[32:35] = c3
    bias65[64] = -cen_w.astype(f64).sum(0)[0]

    sc = np.zeros((C, 11), np.float32)
    sc[:, 0] = b1
    sc[:, 1] = bc
    sc[:, 2] = b2
    sc[0:65, 3] = bias65
    sc[32:35, 4] = mn
    sc[32:35, 5] = mx
    sc[:, 6] = b1 + 1.0
    sc[:, 7] = bc + 1.0
    sc[:, 8] = b2 + 1.0

    nwb = 406 if zero_mask else 660
    wb = np.zeros((C, nwb), BF16)
    wb[:, 0:128] = W1.astype(BF16)
    wb[:, 128:256] = Wc.astype(BF16)
    wb[:, 256:384] = W2.astype(BF16)
    if zero_mask:
        wb[:, 384:402] = sem_w.astype(f64).astype(BF16)
        wb[:, 402:405] = W3.astype(BF16)
        wb[:, 405:406] = cen_w.astype(f64).astype(BF16)
    else:
        # j-major layout: rows [cls 0:18 | cen 18 | regJ 19:127]
        sc64 = scales.astype(f64)
        semJF = np.zeros((C, FOH), f64)
        fohF = np.zeros((C, FOH), f64)
        thrF = np.zeros(FOH, np.float32)
        biasF = np.zeros(FOH, np.float32)
        semJF[:, 0:N_CLS] = sem_w.astype(f64)
        fohF[:, 0:N_CLS] = cls_w.astype(f64)
        thrF[0:N_CLS] = LOGIT_THR - sem_b.astype(f64)
        biasF[0:N_CLS] = cls_b.astype(f64) - cls_w.astype(f64).sum(0)
        fohF[:, N_CLS] = cen_w.astype(f64)[:, 0]
        thrF[N_CLS] = -1e30
        biasF[N_CLS] = -cen_w.astype(f64).sum(0)[0]
        for j in range(N_REG):
            for cc in range(N_CLS):
                r = 19 + j * N_CLS + cc
                semJF[:, r] = sem_w.astype(f64)[:, cc]
                fohF[:, r] = reg_w.astype(f64)[:, j] * sc64[cc]
                thrF[r] = LOGIT_THR - sem_b.astype(f64)[cc]
                biasF[r] = -reg_w.astype(f64)[:, j].sum() * sc64[cc]
        wb[:, 384:511] = semJF.astype(BF16)
        wb[:, 511:638] = fohF.astype(BF16)
        wb[:, 638:641] = W3.astype(BF16)
        wb[:, 641:659] = sem_w.astype(f64).astype(BF16)
        wb[:, 659:660] = cen_w.astype(f64).astype(BF16)
        sc[0:FOH, 9] = thrF
        sc[0:FOH, 10] = biasF

    # ---- transposed, channel-major activations ----
    fT = np.ascontiguousarray(feats.T)
    gTf = np.ascontiguousarray(G.astype(np.float32).T)
    cT = coords_xyz.T.astype(np.float32) * VS

    wts = {"wb": wb, "sc": sc}
    in_maps = []
    for c in range(N_CORES):
        s, e = c * PER_CORE, (c + 1) * PER_CORE
        m = dict(wts)
        m["xT"] = np.ascontiguousarray(fT[:, s:e]).astype(BF16)
        m["gT"] = np.ascontiguousarray(gTf[:, s:e]).astype(BF16)
        m["cvs"] = np.ascontiguousarray(cT[:, s:e]).astype(BF16)
        in_maps.append(m)
    return in_maps, zero_mask


_CACHED = {}

# full path: regJ row 19 + j*18 + c  ->  output col 43 + c*6 + j
_REG_COLS = np.empty(FOH - 19, np.int64)
for _j in range(N_REG):
    for _c in range(N_CLS):
        _REG_COLS[_j * N_CLS + _c] = 43 + _c * N_REG + _j


def _assemble(res, zero_mask, n):
    """Map device outputs to reference layout [n, 151] f32."""
    o = np.zeros((n, OUT_ROWS), np.float32)
    outH = res["outH"][:, :n].astype(np.float32)
    o[:, 0:18] = outH[0:18].T
    o[:, 18:21] = outH[32:35].T
    o[:, 21:24] = res["outW"][:, :n].astype(np.float32).T
    o[:, 24] = outH[64]
    if not zero_mask:
        outB = res["outB"][:, :n].astype(np.float32)
        o[:, 25:43] = outB[0:N_CLS].T
        o[:, _REG_COLS] = outB[19:FOH].T
    return o


def kernel(**inputs):
    inputs = {k: np.asarray(v) for k, v in inputs.items()}
    in_maps, zero_mask = _host_prep(**inputs)
    ckey = "fast" if zero_mask else "full"
    if ckey not in _CACHED:
        _CACHED[ckey] = _build_fast() if zero_mask else _build_full()
    nc = _CACHED[ckey]
    res = run_bass_kernel_spmd(nc, in_maps, core_ids=list(range(N_CORES)))
    out = np.empty((N_VOX, OUT_ROWS), np.float32)
    for c in range(N_CORES):
        out[c * PER_CORE:(c + 1) * PER_CORE] = _assemble(
            res.results[c], zero_mask, PER_CORE)
    return out


# revision 10
# speedup vs baseline: 2.0661x; 1.0006x over previous
"""CAGroup3DHead kernel for 8 Trainium2 NeuronCores.

Strategy (data-parallel over voxels, per the sharding hint):
  - Host: integer index work (sorted-key neighbor lookup identical to the
    reference), weight fusion (BN folded into weights, ELU+1 bias shifts),
    and sharding marshaling (transpose to channel-major, bf16 cast,
    per-core slices).  The 3x3x3 sparse conv collapses to a gather: the
    (0,0,0) tap always hits, so conv_in = feats[rep]; the rare other-tap
    hits are folded into conv_in via W_k @ W_13^{-1} so the device conv is
    one dense matmul.
  - Semantic gate fast path: cls and reg_pc are gated by
    sigmoid(sem) > 0.15, i.e. sem_logit + sem_b > logit(0.15).  The host
    checks the rigorous bound max_i ||feats_i|| * max_c ||sem_w_c|| +
    sem_b_c < logit(0.15); when it holds (it does for the detection-prior
    bias -4.595 used here), every mask entry is exactly zero, so cls and
    reg_pc are exactly zero and the device skips them.  If the bound ever
    fails, a full device path (mask + masked heads) is built instead.
  - Device fast path (identical SPMD program on 8 cores): inputs are
    SBUF-resident (a few large DMAs); work proceeds in 1024-column
    supertiles.  Three 128x128 bf16 matmul chains (mlp1, conv, mlp2) with
    ELU+1 done as one scalar Exp + one relu + one min (exact); the three
    skinny heads (sem 18, voff 3, cen 1) share one [65,T] PSUM tile via
    32-row tile_position strips and drain with a single scalar op.
    Outputs are bf16 and stream out per-supertile; the host re-transposes
    and fills the zero sections during unsharding.
"""

import numpy as np
import ml_dtypes

import concourse.bass as bass
import concourse.bacc as bacc
import concourse.tile as tile
from concourse import mybir
from concourse.bass_utils import run_bass_kernel_spmd

BF16 = ml_dtypes.bfloat16

N_VOX = 100000
C = 128
N_CLS = 18
N_REG = 6
VS = 0.04
THR = 0.15
HASH_D = 260
N_CORES = 8
PER_CORE = N_VOX // N_CORES          # 12500
ST = 1024                            # supertile columns
N_ST = 13                            # 12 full + tail of 212
LOGIT_THR = float(np.log(THR / (1.0 - THR)))   # -1.734601..

FOH = 127                            # full-path head rows [cls|cen|regJ]
OUT_ROWS = 151

F32 = mybir.dt.float32
BF = mybir.dt.bfloat16
AOp = mybir.AluOpType
Act = mybir.ActivationFunctionType


def _build_fast():
    """Fast path: masks provably all-zero; compute sem/voff/voted/cen only."""
    nc = bacc.Bacc(trn_type="TRN2")

    xT_d = nc.dram_tensor("xT", [C, PER_CORE], BF, kind="ExternalInput")
    gT_d = nc.dram_tensor("gT", [C, PER_CORE], BF, kind="ExternalInput")
    cvs_d = nc.dram_tensor("cvs", [3, PER_CORE], BF, kind="ExternalInput")
    # bf16 weights: w1 0:128, wc 128:256, w2 256:384, sem 384:402,
    # w3 402:405, cen 405:406
    wb_d = nc.dram_tensor("wb", [C, 406], BF, kind="ExternalInput")
    # f32 per-partition scalars [128, 11]: col0 b1, col1 bc, col2 b2,
    # col3 bias65 (rows 0:18 semb, 32:35 c3, 64 cenb),
    # col4 mn (rows 32:35), col5 mx (rows 32:35), col6-8 b1/bc/b2 + 1
    sc_d = nc.dram_tensor("sc", [C, 11], F32, kind="ExternalInput")
    outH_d = nc.dram_tensor("outH", [65, PER_CORE], BF, kind="ExternalOutput")
    outW_d = nc.dram_tensor("outW", [3, PER_CORE], BF, kind="ExternalOutput")

    chunks = [(0, 3072), (3072, 6144), (6144, 9216), (9216, PER_CORE)]

    with tile.TileContext(nc) as tc:
        with (
            tc.tile_pool(name="res", bufs=1) as res,
            tc.tile_pool(name="epool", bufs=4) as epool,
            tc.tile_pool(name="rpool", bufs=4) as rpool,
            tc.tile_pool(name="fpool", bufs=4) as fpool,
            tc.tile_pool(name="hpool", bufs=3) as hpool,
            tc.tile_pool(name="pbig", bufs=3, space=bass.MemorySpace.PSUM) as pbig,
            tc.tile_pool(name="phead", bufs=1, space=bass.MemorySpace.PSUM) as ph,
        ):
            wb = res.tile([C, 406], BF)
            sc = res.tile([C, 11], F32)
            nc.sync.dma_start(wb[:], wb_d[:])
            nc.sync.dma_start(sc[:], sc_d[:])
            w1 = wb[:, 0:128]
            wc = wb[:, 128:256]
            w2 = wb[:, 256:384]
            semw = wb[:, 384:402]
            w3 = wb[:, 402:405]
            cenw = wb[:, 405:406]
            b1 = sc[:, 0:1]
            bc = sc[:, 1:2]
            b2 = sc[:, 2:3]
            bias65 = sc[0:65, 3:4]
            mn3 = sc[32:35, 4:5]
            mx3 = sc[32:35, 5:6]
            b1p1 = sc[:, 6:7]
            bcp1 = sc[:, 7:8]
            b2p1 = sc[:, 8:9]

            xT = res.tile([C, PER_CORE], BF)
            gT = res.tile([C, PER_CORE], BF)
            cvsr = res.tile([35, PER_CORE], BF)
            for a, b in chunks:
                nc.sync.dma_start(xT[:, a:b], xT_d[:, a:b])
                nc.sync.dma_start(gT[:, a:b], gT_d[:, a:b])
                nc.sync.dma_start(cvsr[32:35, a:b], cvs_d[:, a:b])

            def mm_blocks(p, w, src_sb, off, L, rows=None, tp=None):
                for a in range(0, L, 512):
                    e = min(a + 512, L)
                    dst = p[:, a:e] if rows is None else p[rows[0]:rows[1], a:e]
                    if tp is None:
                        nc.tensor.matmul(dst, w, src_sb[:, off + a:off + e],
                                         start=True, stop=True)
                    else:
                        nc.tensor.matmul(dst, w, src_sb[:, off + a:off + e],
                                         start=True, stop=True,
                                         tile_position=tp)

            def elu_tail(p, bias, biasp1, L):
                # f = min(max(y+b+1, 1), exp(y+b)) == ELU(y+b) + 1, exact
                et = epool.tile([C, ST], BF, tag="e")
                nc.scalar.activation(et[:, :L], p[:, :L], Act.Exp, bias=bias)
                rt = rpool.tile([C, ST], BF, tag="r")
                nc.vector.tensor_scalar(rt[:, :L], p[:, :L], biasp1, 1.0,
                                        AOp.add, AOp.max)
                ft = fpool.tile([C, ST], BF, tag="f")
                nc.vector.tensor_tensor(ft[:, :L], rt[:, :L], et[:, :L],
                                        AOp.min)
                return ft

            def head_tail(st):
                # finish supertile st: w3 strip, drain, voted chain, stores
                cs0, L0, phd0, f20 = st
                mm_blocks(phd0, w3, f20, 0, L0, rows=(32, 35), tp=(0, 32))
                sv = hpool.tile([65, ST], BF, tag="sv")
                nc.scalar.activation(sv[:, :L0], phd0[:, :L0], Act.Identity,
                                     bias=bias65)
                nc.sync.dma_start(outH_d[:, cs0:cs0 + L0], sv[:, :L0])
                vp = hpool.tile([35, ST], BF, tag="vp")
                nc.gpsimd.tensor_tensor(vp[32:35, :L0], sv[32:35, :L0],
                                        cvsr[32:35, cs0:cs0 + L0], AOp.add)
                vt = hpool.tile([35, ST], BF, tag="vt")
                nc.vector.tensor_scalar(vt[32:35, :L0], vp[32:35, :L0],
                                        mn3, mx3, AOp.max, AOp.min)
                nc.sync.dma_start(outW_d[:, cs0:cs0 + L0], vt[32:35, :L0])

            pending = None
            for k in range(N_ST):
                cs = k * ST
                L = min(ST, PER_CORE - cs)

                # mlp1 + conv matmuls, then sem (xT-only) early for overlap
                p1 = pbig.tile([C, ST], F32, tag="p")
                mm_blocks(p1, w1, xT, cs, L)
                pc = pbig.tile([C, ST], F32, tag="p")
                mm_blocks(pc, wc, gT, cs, L)
                if pending is not None:
                    head_tail(pending)
                phd = ph.tile([65, ST], F32, tag="ph")
                mm_blocks(phd, semw, xT, cs, L, rows=(0, 18), tp=(0, 0))

                f1 = elu_tail(p1, b1, b1p1, L)
                fo = elu_tail(pc, bc, bcp1, L)

                p2 = pbig.tile([C, ST], F32, tag="p")
                mm_blocks(p2, w2, f1, 0, L)
                mm_blocks(phd, cenw, fo, 0, L, rows=(64, 65), tp=(0, 64))
                f2 = elu_tail(p2, b2, b2p1, L)
                pending = (cs, L, phd, f2)

            head_tail(pending)

    nc.finalize()
    return nc


def _build_full():
    """Fallback: full mask + masked-heads path (used if the zero-mask
    bound fails).  j-major reg layout, one is_gt + one STT for heads."""
    nc = bacc.Bacc(trn_type="TRN2")

    xT_d = nc.dram_tensor("xT", [C, PER_CORE], BF, kind="ExternalInput")
    gT_d = nc.dram_tensor("gT", [C, PER_CORE], BF, kind="ExternalInput")
    cvs_d = nc.dram_tensor("cvs", [3, PER_CORE], BF, kind="ExternalInput")
    # w1 0:128, wc 128:256, w2 256:384, semJF 384:511, fohF 511:638,
    # w3 638:641, sem 641:659, cen 659:660
    wb_d = nc.dram_tensor("wb", [C, 660], BF, kind="ExternalInput")
    # col0 b2, col1 bias65, col2 mn, col3 mx, col4 thrF, col5 biasF
    sc_d = nc.dram_tensor("sc", [C, 8], F32, kind="ExternalInput")
    outH_d = nc.dram_tensor("outH", [65, PER_CORE], BF, kind="ExternalOutput")
    outW_d = nc.dram_tensor("outW", [3, PER_CORE], BF, kind="ExternalOutput")
    outB_d = nc.dram_tensor("outB", [FOH, PER_CORE], BF, kind="ExternalOutput")

    chunks = [(0, 3072), (3072, 6144), (6144, 9216), (9216, PER_CORE)]

    with tile.TileContext(nc) as tc:
        with (
            tc.tile_pool(name="res", bufs=1) as res,
            tc.tile_pool(name="epool", bufs=4) as epool,
            tc.tile_pool(name="rpool", bufs=4) as rpool,
            tc.tile_pool(name="fpool", bufs=4) as fpool,
            tc.tile_pool(name="hpool", bufs=3) as hpool,
            tc.tile_pool(name="mpool", bufs=2) as mpool,
            tc.tile_pool(name="pbig", bufs=2, space=bass.MemorySpace.PSUM) as pbig,
            tc.tile_pool(name="phead", bufs=1, space=bass.MemorySpace.PSUM) as ph,
            tc.tile_pool(name="pfoh", bufs=1, space=bass.MemorySpace.PSUM) as pf,
        ):
            wb = res.tile([C, 660], BF)
            sc = res.tile([C, 11], F32)
            nc.sync.dma_start(wb[:], wb_d[:])
            nc.sync.dma_start(sc[:], sc_d[:])
            w1 = wb[:, 0:128]
            wc = wb[:, 128:256]
            w2 = wb[:, 256:384]
            semJF = wb[:, 384:511]
            fohF = wb[:, 511:638]
            w3 = wb[:, 638:641]
            semw = wb[:, 641:659]
            cenw = wb[:, 659:660]
            b1 = sc[:, 0:1]
            bc = sc[:, 1:2]
            b2 = sc[:, 2:3]
            bias65 = sc[0:65, 3:4]
            mn3 = sc[32:35, 4:5]
            mx3 = sc[32:35, 5:6]
            b1p1 = sc[:, 6:7]
            bcp1 = sc[:, 7:8]
            b2p1 = sc[:, 8:9]
            thrF = sc[0:FOH, 9:10]
            biasF = sc[0:FOH, 10:11]

            xT = res.tile([C, PER_CORE], BF)
            gT = res.tile([C, PER_CORE], BF)
            cvsr = res.tile([35, PER_CORE], BF)
            for a, b in chunks:
                nc.sync.dma_start(xT[:, a:b], xT_d[:, a:b])
                nc.sync.dma_start(gT[:, a:b], gT_d[:, a:b])
                nc.sync.dma_start(cvsr[32:35, a:b], cvs_d[:, a:b])

            for k in range(N_ST):
                cs = k * ST
                L = min(ST, PER_CORE - cs)

                def elu_layer(w, src_sb, off, bias):
                    p = pbig.tile([C, ST], F32, tag="p")
                    for a in range(0, L, 512):
                        e = min(a + 512, L)
                        nc.tensor.matmul(p[:, a:e], w,
                                         src_sb[:, off + a:off + e],
                                         start=True, stop=True)
                    et = epool.tile([C, ST], BF, tag="e")
                    nc.scalar.activation(et[:, :L], p[:, :L], Act.Exp,
                                         bias=bias)
                    rt = rpool.tile([C, ST], BF, tag="r")
                    nc.vector.tensor_scalar(rt[:, :L], p[:, :L], bias,
                                            0.0, AOp.add, AOp.max)
                    ft = fpool.tile([C, ST], BF, tag="f")
                    nc.vector.scalar_tensor_tensor(
                        ft[:, :L], rt[:, :L], 1.0, et[:, :L],
                        AOp.add, AOp.min)
                    return ft

                f1 = elu_layer(w1, xT, cs, b1)
                fo = elu_layer(wc, gT, cs, bc)
                f2 = elu_layer(w2, f1, 0, b2)

                phd = ph.tile([65, ST], F32, tag="ph")
                for a in range(0, L, 512):
                    e = min(a + 512, L)
                    nc.tensor.matmul(phd[0:18, a:e], semw,
                                     xT[:, cs + a:cs + e],
                                     start=True, stop=True,
                                     tile_position=(0, 0))
                    nc.tensor.matmul(phd[32:35, a:e], w3, f2[:, a:e],
                                     start=True, stop=True,
                                     tile_position=(0, 32))
                    nc.tensor.matmul(phd[64:65, a:e], cenw, fo[:, a:e],
                                     start=True, stop=True,
                                     tile_position=(0, 64))
                sv = hpool.tile([65, ST], BF, tag="sv")
                nc.scalar.activation(sv[:, :L], phd[:, :L], Act.Identity,
                                     bias=bias65)
                nc.sync.dma_start(outH_d[:, cs:cs + L], sv[:, :L])

                vp = hpool.tile([35, ST], BF, tag="vp")
                nc.gpsimd.tensor_tensor(vp[32:35, :L], sv[32:35, :L],
                                        cvsr[32:35, cs:cs + L], AOp.add)
                vt = hpool.tile([35, ST], BF, tag="vt")
                nc.vector.tensor_scalar(vt[32:35, :L], vp[32:35, :L],
                                        mn3, mx3, AOp.max, AOp.min)
                nc.sync.dma_start(outW_d[:, cs:cs + L], vt[32:35, :L])

                # masked heads: (foh + biasF) * (semJ > thrF)
                psem = ph.tile([FOH, ST], F32, tag="ph")
                for a in range(0, L, 512):
                    e = min(a + 512, L)
                    nc.tensor.matmul(psem[:, a:e], semJF,
                                     xT[:, cs + a:cs + e],
                                     start=True, stop=True)
                maskF = mpool.tile([FOH, ST], BF, tag="maskF")
                nc.vector.tensor_scalar(maskF[:, :L], psem[:, :L], thrF,
                                        None, AOp.is_gt)
                pfo = pf.tile([FOH, ST], F32, tag="pfoh")
                for a in range(0, L, 512):
                    e = min(a + 512, L)
                    nc.tensor.matmul(pfo[:, a:e], fohF, fo[:, a:e],
                                     start=True, stop=True)
                hb = mpool.tile([FOH, ST], BF, tag="hb")
                nc.vector.scalar_tensor_tensor(
                    hb[:, :L], pfo[:, :L], biasF, maskF[:, :L],
                    AOp.add, AOp.mult)
                nc.sync.dma_start(outB_d[:, cs:cs + L], hb[:, :L])

    nc.finalize()
    return nc


def _host_prep(feats, coords_xyz, batch_idx,
               off_w1, off_g1, off_b1, off_w2, off_g2, off_b2, off_w3,
               fo_w, fo_g, fo_b, sem_w, sem_b, cen_w, cls_w, cls_b, reg_w,
               scales):
    f64 = np.float64
    N = feats.shape[0]

    # ---- zero-mask bound: |x.w_c| <= max||x|| * ||w_c|| (Cauchy-Schwarz)
    feats64 = feats.astype(f64)
    max_row = np.sqrt((feats64 * feats64).sum(1).max())
    colnrm = np.sqrt((sem_w.astype(f64) ** 2).sum(0))
    zero_mask = bool(
        np.all(max_row * colnrm + sem_b.astype(f64) < LOGIT_THR - 1e-3))

    # ---- neighbor lookup (identical to reference's sorted-key search) ----
    c1 = coords_xyz.astype(np.int64) + 1
    key = ((batch_idx.astype(np.int64) * HASH_D + c1[:, 0]) * HASH_D
           + c1[:, 1]) * HASH_D + c1[:, 2]
    order = np.argsort(key, kind="stable")
    skey = key[order]
    pos = np.searchsorted(skey, key)
    rep = order[pos]                      # first voxel with same key

    # ---- fused weights (BN folded; ELU+1 handled via bias shifts) ----
    W1 = off_w1.astype(f64) * off_g1.astype(f64)[None, :]
    b1 = off_b1.astype(f64)
    W2 = off_w2.astype(f64) * off_g2.astype(f64)[None, :]
    b2 = off_b2.astype(f64) - W2.sum(0)
    W3 = off_w3.astype(f64)
    c3 = -W3.sum(0)
    Wc = fo_w[13].astype(f64) * fo_g.astype(f64)[None, :]
    bc = fo_b.astype(f64)

    # ---- conv input: gather + fold rare non-center taps via Wc13^-1 ----
    G = feats64[rep]
    Winv = np.linalg.inv(fo_w[13].astype(f64))
    k = 0
    for dx in (-1, 0, 1):
        for dy in (-1, 0, 1):
            for dz in (-1, 0, 1):
                if (dx, dy, dz) != (0, 0, 0):
                    nk = key + (dx * HASH_D + dy) * HASH_D + dz
                    p = np.clip(np.searchsorted(skey, nk), 0, N - 1)
                    hit = skey[p] == nk
                    if hit.any():
                        dst = np.nonzero(hit)[0]
                        src = order[p[hit]]
                        A = fo_w[k].astype(f64) @ Winv
                        np.add.at(G, dst, feats64[src] @ A)
                k += 1

    # ---- shared scalar columns ----
    mx = (coords_xyz.max(0) + 1).astype(f64) * VS
    mn = (coords_xyz.min(0) - 1).astype(f64) * VS
    bias65 = np.zeros(65, f64)
    bias65[0:N_CLS] = sem_b.astype(f64)
    bias65[32:35] = c3
    bias65[64] = -cen_w.astype(f64).sum(0)[0]

    sc = np.zeros((C, 11), np.float32)
    sc[:, 0] = b1
    sc[:, 1] = bc
    sc[:, 2] = b2
    sc[0:65, 3] = bias65
    sc[32:35, 4] = mn
    sc[32:35, 5] = mx
    sc[:, 6] = b1 + 1.0
    sc[:, 7] = bc + 1.0
    sc[:, 8] = b2 + 1.0

    nwb = 406 if zero_mask else 660
    wb = np.zeros((C, nwb), BF16)
    wb[:, 0:128] = W1.astype(BF16)
    wb[:, 128:256] = Wc.astype(BF16)
    wb[:, 256:384] = W2.astype(BF16)
    if zero_mask:
        wb[:, 384:402] = sem_w.astype(f64).astype(BF16)
        wb[:, 402:405] = W3.astype(BF16)
        wb[:, 405:406] = cen_w.astype(f64).astype(BF16)
    else:
        # j-major layout: rows [cls 0:18 | cen 18 | regJ 19:127]
        sc64 = scales.astype(f64)
        semJF = np.zeros((C, FOH), f64)
        fohF = np.zeros((C, FOH), f64)
        thrF = np.zeros(FOH, np.float32)
        biasF = np.zeros(FOH, np.float32)
        semJF[:, 0:N_CLS] = sem_w.astype(f64)
        fohF[:, 0:N_CLS] = cls_w.astype(f64)
        thrF[0:N_CLS] = LOGIT_THR - sem_b.astype(f64)
        biasF[0:N_CLS] = cls_b.astype(f64) - cls_w.astype(f64).sum(0)
        fohF[:, N_CLS] = cen_w.astype(f64)[:, 0]
        thrF[N_CLS] = -1e30
        biasF[N_CLS] = -cen_w.astype(f64).sum(0)[0]
        for j in range(N_REG):
            for cc in range(N_CLS):
                r = 19 + j * N_CLS + cc
                semJF[:, r] = sem_w.astype(f64)[:, cc]
                fohF[:, r] = reg_w.astype(f64)[:, j] * sc64[cc]
                thrF[r] = LOGIT_THR - sem_b.astype(f64)[cc]
                biasF[r] = -reg_w.astype(f64)[:, j].sum() * sc64[cc]
        wb[:, 384:511] = semJF.astype(BF16)
        wb[:, 511:638] = fohF.astype(BF16)
        wb[:, 638:641] = W3.astype(BF16)
        wb[:, 641:659] = sem_w.astype(f64).astype(BF16)
        wb[:, 659:660] = cen_w.astype(f64).astype(BF16)
        sc[0:FOH, 9] = thrF
        sc[0:FOH, 10] = biasF

    # ---- transposed, channel-major activations ----
    fT = np.ascontiguousarray(feats.T)
    gTf = np.ascontiguousarray(G.astype(np.float32).T)
    cT = coords_xyz.T.astype(np.float32) * VS

    wts = {"wb": wb, "sc": sc}
    in_maps = []
    for c in range(N_CORES):
        s, e = c * PER_CORE, (c + 1) * PER_CORE
        m = dict(wts)
        m["xT"] = np.ascontiguousarray(fT[:, s:e]).astype(BF16)
        m["gT"] = np.ascontiguousarray(gTf[:, s:e]).astype(BF16)
        m["cvs"] = np.ascontiguousarray(cT[:, s:e]).astype(BF16)
        in_maps.append(m)
    return in_maps, zero_mask


_CACHED = {}

# full path: regJ row 19 + j*18 + c  ->  output col 43 + c*6 + j
_REG_COLS = np.empty(FOH - 19, np.int64)
for _j in range(N_REG):
    for _c in range(N_CLS):
        _REG_COLS[_j * N_CLS + _c] = 43 + _c * N_REG + _j


def _assemble(res, zero_mask, n):
    """Map device outputs to reference layout [n, 151] f32."""
    o = np.zeros((n, OUT_ROWS), np.float32)
    outH = res["outH"][:, :n].astype(np.float32)
    o[:, 0:18] = outH[0:18].T
    o[:, 18:21] = outH[32:35].T
    o[:, 21:24] = res["outW"][:, :n].astype(np.float32).T
    o[:, 24] = outH[64]
    if not zero_mask:
        outB = res["outB"][:, :n].astype(np.float32)
        o[:, 25:43] = outB[0:N_CLS].T
        o[:, _REG_COLS] = outB[19:FOH].T
    return o


def kernel(**inputs):
    inputs = {k: np.asarray(v) for k, v in inputs.items()}
    in_maps, zero_mask = _host_prep(**inputs)
    ckey = "fast" if zero_mask else "full"
    if ckey not in _CACHED:
        _CACHED[ckey] = _build_fast() if zero_mask else _build_full()
    nc = _CACHED[ckey]
    res = run_bass_kernel_spmd(nc, in_maps, core_ids=list(range(N_CORES)))
    out = np.empty((N_VOX, OUT_ROWS), np.float32)
    for c in range(N_CORES):
        out[c * PER_CORE:(c + 1) * PER_CORE] = _assemble(
            res.results[c], zero_mask, PER_CORE)
    return out


# revision 11
# speedup vs baseline: 2.1156x; 1.0240x over previous
"""CAGroup3DHead kernel for 8 Trainium2 NeuronCores.

Strategy (data-parallel over voxels, per the sharding hint):
  - Host: integer index work (sorted-key neighbor lookup identical to the
    reference), weight fusion (BN folded into weights, ELU+1 bias shifts),
    and sharding marshaling (transpose to channel-major, bf16 cast,
    per-core slices).  The 3x3x3 sparse conv collapses to a gather: the
    (0,0,0) tap always hits, so conv_in = feats[rep]; the rare other-tap
    hits are folded into conv_in via W_k @ W_13^{-1} so the device conv is
    one dense matmul.
  - Semantic gate fast path: cls and reg_pc are gated by
    sigmoid(sem) > 0.15, i.e. sem_logit + sem_b > logit(0.15).  The host
    checks the rigorous bound max_i ||feats_i|| * max_c ||sem_w_c|| +
    sem_b_c < logit(0.15); when it holds (it does for the detection-prior
    bias -4.595 used here), every mask entry is exactly zero, so cls and
    reg_pc are exactly zero and the device skips them.  If the bound ever
    fails, a full device path (mask + masked heads) is built instead.
  - Device fast path (identical SPMD program on 8 cores): inputs are
    SBUF-resident (a few large DMAs); work proceeds in 1024-column
    supertiles.  Three 128x128 bf16 matmul chains (mlp1, conv, mlp2) with
    ELU+1 done as one scalar Exp + one relu + one min (exact); the three
    skinny heads (sem 18, voff 3, cen 1) share one [65,T] PSUM tile via
    32-row tile_position strips and drain with a single scalar op.
    Outputs are bf16 and stream out per-supertile; the host re-transposes
    and fills the zero sections during unsharding.
"""

import numpy as np
import ml_dtypes

import concourse.bass as bass
import concourse.bacc as bacc
import concourse.tile as tile
from concourse import mybir
from concourse.bass_utils import run_bass_kernel_spmd

BF16 = ml_dtypes.bfloat16

N_VOX = 100000
C = 128
N_CLS = 18
N_REG = 6
VS = 0.04
THR = 0.15
HASH_D = 260
N_CORES = 8
PER_CORE = N_VOX // N_CORES          # 12500
ST = 1024                            # supertile columns
N_ST = 13                            # 12 full + tail of 212
LOGIT_THR = float(np.log(THR / (1.0 - THR)))   # -1.734601..

FOH = 127                            # full-path head rows [cls|cen|regJ]
OUT_ROWS = 151

F32 = mybir.dt.float32
BF = mybir.dt.bfloat16
AOp = mybir.AluOpType
Act = mybir.ActivationFunctionType


def _build_fast():
    """Fast path: masks provably all-zero; compute sem/voff/voted/cen only."""
    nc = bacc.Bacc(trn_type="TRN2")

    xT_d = nc.dram_tensor("xT", [C, PER_CORE], BF, kind="ExternalInput")
    gT_d = nc.dram_tensor("gT", [C, PER_CORE], BF, kind="ExternalInput")
    cvs_d = nc.dram_tensor("cvs", [3, PER_CORE], BF, kind="ExternalInput")
    # bf16 weights: w1 0:128, wc 128:256, w2 256:384, sem 384:402,
    # w3 402:405, cen 405:406
    wb_d = nc.dram_tensor("wb", [C, 406], BF, kind="ExternalInput")
    # f32 per-partition scalars [128, 11]: col0 b1, col1 bc, col2 b2,
    # col3 bias65 (rows 0:18 semb, 32:35 c3, 64 cenb),
    # col4 mn (rows 32:35), col5 mx (rows 32:35), col6-8 b1/bc/b2 + 1
    sc_d = nc.dram_tensor("sc", [C, 11], F32, kind="ExternalInput")
    outH_d = nc.dram_tensor("outH", [65, PER_CORE], BF, kind="ExternalOutput")
    outW_d = nc.dram_tensor("outW", [3, PER_CORE], BF, kind="ExternalOutput")

    chunks = [(0, 3072), (3072, 6144), (6144, 9216), (9216, PER_CORE)]

    with tile.TileContext(nc) as tc:
        with (
            tc.tile_pool(name="res", bufs=1) as res,
            tc.tile_pool(name="epool", bufs=4) as epool,
            tc.tile_pool(name="rpool", bufs=4) as rpool,
            tc.tile_pool(name="fpool", bufs=4) as fpool,
            tc.tile_pool(name="hpool", bufs=3) as hpool,
            tc.tile_pool(name="pbig", bufs=3, space=bass.MemorySpace.PSUM) as pbig,
            tc.tile_pool(name="phead", bufs=1, space=bass.MemorySpace.PSUM) as ph,
        ):
            wb = res.tile([C, 406], BF)
            sc = res.tile([C, 11], F32)
            nc.sync.dma_start(wb[:], wb_d[:])
            nc.sync.dma_start(sc[:], sc_d[:])
            w1 = wb[:, 0:128]
            wc = wb[:, 128:256]
            w2 = wb[:, 256:384]
            semw = wb[:, 384:402]
            w3 = wb[:, 402:405]
            cenw = wb[:, 405:406]
            b1 = sc[:, 0:1]
            bc = sc[:, 1:2]
            b2 = sc[:, 2:3]
            bias65 = sc[0:65, 3:4]
            mn3 = sc[32:35, 4:5]
            mx3 = sc[32:35, 5:6]
            b1p1 = sc[:, 6:7]
            bcp1 = sc[:, 7:8]
            b2p1 = sc[:, 8:9]

            xT = res.tile([C, PER_CORE], BF)
            gT = res.tile([C, PER_CORE], BF)
            cvsr = res.tile([35, PER_CORE], BF)
            for a, b in chunks:
                nc.sync.dma_start(xT[:, a:b], xT_d[:, a:b])
                nc.sync.dma_start(gT[:, a:b], gT_d[:, a:b])
                nc.sync.dma_start(cvsr[32:35, a:b], cvs_d[:, a:b])

            def mm_blocks(p, w, src_sb, off, L, rows=None, tp=None):
                for a in range(0, L, 512):
                    e = min(a + 512, L)
                    dst = p[:, a:e] if rows is None else p[rows[0]:rows[1], a:e]
                    if tp is None:
                        nc.tensor.matmul(dst, w, src_sb[:, off + a:off + e],
                                         start=True, stop=True)
                    else:
                        nc.tensor.matmul(dst, w, src_sb[:, off + a:off + e],
                                         start=True, stop=True,
                                         tile_position=tp)

            def elu_tail(p, bias, biasp1, L, scalar_relu=False):
                # f = min(max(y+b+1, 1), exp(y+b)) == ELU(y+b) + 1, exact
                et = epool.tile([C, ST], BF, tag="e")
                nc.scalar.activation(et[:, :L], p[:, :L], Act.Exp, bias=bias)
                ft = fpool.tile([C, ST], BF, tag="f")
                if scalar_relu:
                    # relu on Scalar, (r+1) min e on Vector (STT)
                    rt = rpool.tile([C, ST], BF, tag="r")
                    nc.scalar.activation(rt[:, :L], p[:, :L], Act.Relu,
                                         bias=bias)
                    nc.vector.scalar_tensor_tensor(
                        ft[:, :L], rt[:, :L], 1.0, et[:, :L],
                        AOp.add, AOp.min)
                else:
                    # max(y+b+1, 1) on Vector (TS), min on Vector (TT, 2x)
                    rt = rpool.tile([C, ST], BF, tag="r")
                    nc.vector.tensor_scalar(rt[:, :L], p[:, :L], biasp1, 1.0,
                                            AOp.add, AOp.max)
                    nc.vector.tensor_tensor(ft[:, :L], rt[:, :L], et[:, :L],
                                            AOp.min)
                return ft

            def head_tail(st):
                # finish supertile st: w3 strip, drain, voted chain, stores
                cs0, L0, phd0, f20 = st
                mm_blocks(phd0, w3, f20, 0, L0, rows=(32, 35), tp=(0, 32))
                sv = hpool.tile([65, ST], BF, tag="sv")
                nc.scalar.activation(sv[:, :L0], phd0[:, :L0], Act.Identity,
                                     bias=bias65)
                nc.sync.dma_start(outH_d[:, cs0:cs0 + L0], sv[:, :L0])
                vp = hpool.tile([35, ST], BF, tag="vp")
                nc.gpsimd.tensor_tensor(vp[32:35, :L0], sv[32:35, :L0],
                                        cvsr[32:35, cs0:cs0 + L0], AOp.add)
                vt = hpool.tile([35, ST], BF, tag="vt")
                nc.vector.tensor_scalar(vt[32:35, :L0], vp[32:35, :L0],
                                        mn3, mx3, AOp.max, AOp.min)
                nc.sync.dma_start(outW_d[:, cs0:cs0 + L0], vt[32:35, :L0])

            pending = None
            for k in range(N_ST):
                cs = k * ST
                L = min(ST, PER_CORE - cs)

                # mlp1 + conv matmuls, then sem (xT-only) early for overlap
                p1 = pbig.tile([C, ST], F32, tag="p")
                mm_blocks(p1, w1, xT, cs, L)
                pc = pbig.tile([C, ST], F32, tag="p")
                mm_blocks(pc, wc, gT, cs, L)
                if pending is not None:
                    head_tail(pending)
                phd = ph.tile([65, ST], F32, tag="ph")
                mm_blocks(phd, semw, xT, cs, L, rows=(0, 18), tp=(0, 0))

                f1 = elu_tail(p1, b1, b1p1, L)
                fo = elu_tail(pc, bc, bcp1, L, scalar_relu=True)

                p2 = pbig.tile([C, ST], F32, tag="p")
                mm_blocks(p2, w2, f1, 0, L)
                mm_blocks(phd, cenw, fo, 0, L, rows=(64, 65), tp=(0, 64))
                f2 = elu_tail(p2, b2, b2p1, L)
                pending = (cs, L, phd, f2)

            head_tail(pending)

    nc.finalize()
    return nc


def _build_full():
    """Fallback: full mask + masked-heads path (used if the zero-mask
    bound fails).  j-major reg layout, one is_gt + one STT for heads."""
    nc = bacc.Bacc(trn_type="TRN2")

    xT_d = nc.dram_tensor("xT", [C, PER_CORE], BF, kind="ExternalInput")
    gT_d = nc.dram_tensor("gT", [C, PER_CORE], BF, kind="ExternalInput")
    cvs_d = nc.dram_tensor("cvs", [3, PER_CORE], BF, kind="ExternalInput")
    # w1 0:128, wc 128:256, w2 256:384, semJF 384:511, fohF 511:638,
    # w3 638:641, sem 641:659, cen 659:660
    wb_d = nc.dram_tensor("wb", [C, 660], BF, kind="ExternalInput")
    # col0 b2, col1 bias65, col2 mn, col3 mx, col4 thrF, col5 biasF
    sc_d = nc.dram_tensor("sc", [C, 8], F32, kind="ExternalInput")
    outH_d = nc.dram_tensor("outH", [65, PER_CORE], BF, kind="ExternalOutput")
    outW_d = nc.dram_tensor("outW", [3, PER_CORE], BF, kind="ExternalOutput")
    outB_d = nc.dram_tensor("outB", [FOH, PER_CORE], BF, kind="ExternalOutput")

    chunks = [(0, 3072), (3072, 6144), (6144, 9216), (9216, PER_CORE)]

    with tile.TileContext(nc) as tc:
        with (
            tc.tile_pool(name="res", bufs=1) as res,
            tc.tile_pool(name="epool", bufs=4) as epool,
            tc.tile_pool(name="rpool", bufs=4) as rpool,
            tc.tile_pool(name="fpool", bufs=4) as fpool,
            tc.tile_pool(name="hpool", bufs=3) as hpool,
            tc.tile_pool(name="mpool", bufs=2) as mpool,
            tc.tile_pool(name="pbig", bufs=2, space=bass.MemorySpace.PSUM) as pbig,
            tc.tile_pool(name="phead", bufs=1, space=bass.MemorySpace.PSUM) as ph,
            tc.tile_pool(name="pfoh", bufs=1, space=bass.MemorySpace.PSUM) as pf,
        ):
            wb = res.tile([C, 660], BF)
            sc = res.tile([C, 11], F32)
            nc.sync.dma_start(wb[:], wb_d[:])
            nc.sync.dma_start(sc[:], sc_d[:])
            w1 = wb[:, 0:128]
            wc = wb[:, 128:256]
            w2 = wb[:, 256:384]
            semJF = wb[:, 384:511]
            fohF = wb[:, 511:638]
            w3 = wb[:, 638:641]
            semw = wb[:, 641:659]
            cenw = wb[:, 659:660]
            b1 = sc[:, 0:1]
            bc = sc[:, 1:2]
            b2 = sc[:, 2:3]
            bias65 = sc[0:65, 3:4]
            mn3 = sc[32:35, 4:5]
            mx3 = sc[32:35, 5:6]
            b1p1 = sc[:, 6:7]
            bcp1 = sc[:, 7:8]
            b2p1 = sc[:, 8:9]
            thrF = sc[0:FOH, 9:10]
            biasF = sc[0:FOH, 10:11]

            xT = res.tile([C, PER_CORE], BF)
            gT = res.tile([C, PER_CORE], BF)
            cvsr = res.tile([35, PER_CORE], BF)
            for a, b in chunks:
                nc.sync.dma_start(xT[:, a:b], xT_d[:, a:b])
                nc.sync.dma_start(gT[:, a:b], gT_d[:, a:b])
                nc.sync.dma_start(cvsr[32:35, a:b], cvs_d[:, a:b])

            for k in range(N_ST):
                cs = k * ST
                L = min(ST, PER_CORE - cs)

                def elu_layer(w, src_sb, off, bias):
                    p = pbig.tile([C, ST], F32, tag="p")
                    for a in range(0, L, 512):
                        e = min(a + 512, L)
                        nc.tensor.matmul(p[:, a:e], w,
                                         src_sb[:, off + a:off + e],
                                         start=True, stop=True)
                    et = epool.tile([C, ST], BF, tag="e")
                    nc.scalar.activation(et[:, :L], p[:, :L], Act.Exp,
                                         bias=bias)
                    rt = rpool.tile([C, ST], BF, tag="r")
                    nc.vector.tensor_scalar(rt[:, :L], p[:, :L], bias,
                                            0.0, AOp.add, AOp.max)
                    ft = fpool.tile([C, ST], BF, tag="f")
                    nc.vector.scalar_tensor_tensor(
                        ft[:, :L], rt[:, :L], 1.0, et[:, :L],
                        AOp.add, AOp.min)
                    return ft

                f1 = elu_layer(w1, xT, cs, b1)
                fo = elu_layer(wc, gT, cs, bc)
                f2 = elu_layer(w2, f1, 0, b2)

                phd = ph.tile([65, ST], F32, tag="ph")
                for a in range(0, L, 512):
                    e = min(a + 512, L)
                    nc.tensor.matmul(phd[0:18, a:e], semw,
                                     xT[:, cs + a:cs + e],
                                     start=True, stop=True,
                                     tile_position=(0, 0))
                    nc.tensor.matmul(phd[32:35, a:e], w3, f2[:, a:e],
                                     start=True, stop=True,
                                     tile_position=(0, 32))
                    nc.tensor.matmul(phd[64:65, a:e], cenw, fo[:, a:e],
                                     start=True, stop=True,
                                     tile_position=(0, 64))
                sv = hpool.tile([65, ST], BF, tag="sv")
                nc.scalar.activation(sv[:, :L], phd[:, :L], Act.Identity,
                                     bias=bias65)
                nc.sync.dma_start(outH_d[:, cs:cs + L], sv[:, :L])

                vp = hpool.tile([35, ST], BF, tag="vp")
                nc.gpsimd.tensor_tensor(vp[32:35, :L], sv[32:35, :L],
                                        cvsr[32:35, cs:cs + L], AOp.add)
                vt = hpool.tile([35, ST], BF, tag="vt")
                nc.vector.tensor_scalar(vt[32:35, :L], vp[32:35, :L],
                                        mn3, mx3, AOp.max, AOp.min)
                nc.sync.dma_start(outW_d[:, cs:cs + L], vt[32:35, :L])

                # masked heads: (foh + biasF) * (semJ > thrF)
                psem = ph.tile([FOH, ST], F32, tag="ph")
                for a in range(0, L, 512):
                    e = min(a + 512, L)
                    nc.tensor.matmul(psem[:, a:e], semJF,
                                     xT[:, cs + a:cs + e],
                                     start=True, stop=True)
                maskF = mpool.tile([FOH, ST], BF, tag="maskF")
                nc.vector.tensor_scalar(maskF[:, :L], psem[:, :L], thrF,
                                        None, AOp.is_gt)
                pfo = pf.tile([FOH, ST], F32, tag="pfoh")
                for a in range(0, L, 512):
                    e = min(a + 512, L)
                    nc.tensor.matmul(pfo[:, a:e], fohF, fo[:, a:e],
                                     start=True, stop=True)
                hb = mpool.tile([FOH, ST], BF, tag="hb")
                nc.vector.scalar_tensor_tensor(
                    hb[:, :L], pfo[:, :L], biasF, maskF[:, :L],
                    AOp.add, AOp.mult)
                nc.sync.dma_start(outB_d[:, cs:cs + L], hb[:, :L])

    nc.finalize()
    return nc


def _host_prep(feats, coords_xyz, batch_idx,
               off_w1, off_g1, off_b1, off_w2, off_g2, off_b2, off_w3,
               fo_w, fo_g, fo_b, sem_w, sem_b, cen_w, cls_w, cls_b, reg_w,
               scales):
    f64 = np.float64
    N = feats.shape[0]

    # ---- zero-mask bound: |x.w_c| <= max||x|| * ||w_c|| (Cauchy-Schwarz)
    feats64 = feats.astype(f64)
    max_row = np.sqrt((feats64 * feats64).sum(1).max())
    colnrm = np.sqrt((sem_w.astype(f64) ** 2).sum(0))
    zero_mask = bool(
        np.all(max_row * colnrm + sem_b.astype(f64) < LOGIT_THR - 1e-3))

    # ---- neighbor lookup (identical to reference's sorted-key search) ----
    c1 = coords_xyz.astype(np.int64) + 1
    key = ((batch_idx.astype(np.int64) * HASH_D + c1[:, 0]) * HASH_D
           + c1[:, 1]) * HASH_D + c1[:, 2]
    order = np.argsort(key, kind="stable")
    skey = key[order]
    pos = np.searchsorted(skey, key)
    rep = order[pos]                      # first voxel with same key

    # ---- fused weights (BN folded; ELU+1 handled via bias shifts) ----
    W1 = off_w1.astype(f64) * off_g1.astype(f64)[None, :]
    b1 = off_b1.astype(f64)
    W2 = off_w2.astype(f64) * off_g2.astype(f64)[None, :]
    b2 = off_b2.astype(f64) - W2.sum(0)
    W3 = off_w3.astype(f64)
    c3 = -W3.sum(0)
    Wc = fo_w[13].astype(f64) * fo_g.astype(f64)[None, :]
    bc = fo_b.astype(f64)

    # ---- conv input: gather + fold rare non-center taps via Wc13^-1 ----
    G = feats64[rep]
    Winv = np.linalg.inv(fo_w[13].astype(f64))
    k = 0
    for dx in (-1, 0, 1):
        for dy in (-1, 0, 1):
            for dz in (-1, 0, 1):
                if (dx, dy, dz) != (0, 0, 0):
                    nk = key + (dx * HASH_D + dy) * HASH_D + dz
                    p = np.clip(np.searchsorted(skey, nk), 0, N - 1)
                    hit = skey[p] == nk
                    if hit.any():
                        dst = np.nonzero(hit)[0]
                        src = order[p[hit]]
                        A = fo_w[k].astype(f64) @ Winv
                        np.add.at(G, dst, feats64[src] @ A)
                k += 1

    # ---- shared scalar columns ----
    mx = (coords_xyz.max(0) + 1).astype(f64) * VS
    mn = (coords_xyz.min(0) - 1).astype(f64) * VS
    bias65 = np.zeros(65, f64)
    bias65[0:N_CLS] = sem_b.astype(f64)
    bias65[32:35] = c3
    bias65[64] = -cen_w.astype(f64).sum(0)[0]

    sc = np.zeros((C, 11), np.float32)
    sc[:, 0] = b1
    sc[:, 1] = bc
    sc[:, 2] = b2
    sc[0:65, 3] = bias65
    sc[32:35, 4] = mn
    sc[32:35, 5] = mx
    sc[:, 6] = b1 + 1.0
    sc[:, 7] = bc + 1.0
    sc[:, 8] = b2 + 1.0

    nwb = 406 if zero_mask else 660
    wb = np.zeros((C, nwb), BF16)
    wb[:, 0:128] = W1.astype(BF16)
    wb[:, 128:256] = Wc.astype(BF16)
    wb[:, 256:384] = W2.astype(BF16)
    if zero_mask:
        wb[:, 384:402] = sem_w.astype(f64).astype(BF16)
        wb[:, 402:405] = W3.astype(BF16)
        wb[:, 405:406] = cen_w.astype(f64).astype(BF16)
    else:
        # j-major layout: rows [cls 0:18 | cen 18 | regJ 19:127]
        sc64 = scales.astype(f64)
        semJF = np.zeros((C, FOH), f64)
        fohF = np.zeros((C, FOH), f64)
        thrF = np.zeros(FOH, np.float32)
        biasF = np.zeros(FOH, np.float32)
        semJF[:, 0:N_CLS] = sem_w.astype(f64)
        fohF[:, 0:N_CLS] = cls_w.astype(f64)
        thrF[0:N_CLS] = LOGIT_THR - sem_b.astype(f64)
        biasF[0:N_CLS] = cls_b.astype(f64) - cls_w.astype(f64).sum(0)
        fohF[:, N_CLS] = cen_w.astype(f64)[:, 0]
        thrF[N_CLS] = -1e30
        biasF[N_CLS] = -cen_w.astype(f64).sum(0)[0]
        for j in range(N_REG):
            for cc in range(N_CLS):
                r = 19 + j * N_CLS + cc
                semJF[:, r] = sem_w.astype(f64)[:, cc]
                fohF[:, r] = reg_w.astype(f64)[:, j] * sc64[cc]
                thrF[r] = LOGIT_THR - sem_b.astype(f64)[cc]
                biasF[r] = -reg_w.astype(f64)[:, j].sum() * sc64[cc]
        wb[:, 384:511] = semJF.astype(BF16)
        wb[:, 511:638] = fohF.astype(BF16)
        wb[:, 638:641] = W3.astype(BF16)
        wb[:, 641:659] = sem_w.astype(f64).astype(BF16)
        wb[:, 659:660] = cen_w.astype(f64).astype(BF16)
        sc[0:FOH, 9] = thrF
        sc[0:FOH, 10] = biasF

    # ---- transposed, channel-major activations ----
    fT = np.ascontiguousarray(feats.T)
    gTf = np.ascontiguousarray(G.astype(np.float32).T)
    cT = coords_xyz.T.astype(np.float32) * VS

    wts = {"wb": wb, "sc": sc}
    in_maps = []
    for c in range(N_CORES):
        s, e = c * PER_CORE, (c + 1) * PER_CORE
        m = dict(wts)
        m["xT"] = np.ascontiguousarray(fT[:, s:e]).astype(BF16)
        m["gT"] = np.ascontiguousarray(gTf[:, s:e]).astype(BF16)
        m["cvs"] = np.ascontiguousarray(cT[:, s:e]).astype(BF16)
        in_maps.append(m)
    return in_maps, zero_mask


_CACHED = {}

# full path: regJ row 19 + j*18 + c  ->  output col 43 + c*6 + j
_REG_COLS = np.empty(FOH - 19, np.int64)
for _j in range(N_REG):
    for _c in range(N_CLS):
        _REG_COLS[_j * N_CLS + _c] = 43 + _c * N_REG + _j


def _assemble(res, zero_mask, n):
    """Map device outputs to reference layout [n, 151] f32."""
    o = np.zeros((n, OUT_ROWS), np.float32)
    outH = res["outH"][:, :n].astype(np.float32)
    o[:, 0:18] = outH[0:18].T
    o[:, 18:21] = outH[32:35].T
    o[:, 21:24] = res["outW"][:, :n].astype(np.float32).T
    o[:, 24] = outH[64]
    if not zero_mask:
        outB = res["outB"][:, :n].astype(np.float32)
        o[:, 25:43] = outB[0:N_CLS].T
        o[:, _REG_COLS] = outB[19:FOH].T
    return o


def kernel(**inputs):
    inputs = {k: np.asarray(v) for k, v in inputs.items()}
    in_maps, zero_mask = _host_prep(**inputs)
    ckey = "fast" if zero_mask else "full"
    if ckey not in _CACHED:
        _CACHED[ckey] = _build_fast() if zero_mask else _build_full()
    nc = _CACHED[ckey]
    res = run_bass_kernel_spmd(nc, in_maps, core_ids=list(range(N_CORES)))
    out = np.empty((N_VOX, OUT_ROWS), np.float32)
    for c in range(N_CORES):
        out[c * PER_CORE:(c + 1) * PER_CORE] = _assemble(
            res.results[c], zero_mask, PER_CORE)
    return out


# revision 12
# speedup vs baseline: 2.1252x; 1.0045x over previous
"""CAGroup3DHead kernel for 8 Trainium2 NeuronCores.

Strategy (data-parallel over voxels, per the sharding hint):
  - Host: integer index work (sorted-key neighbor lookup identical to the
    reference), weight fusion (BN folded into weights, ELU+1 bias shifts),
    and sharding marshaling (transpose to channel-major, bf16 cast,
    per-core slices).  The 3x3x3 sparse conv collapses to a gather: the
    (0,0,0) tap always hits, so conv_in = feats[rep]; the rare other-tap
    hits are folded into conv_in via W_k @ W_13^{-1} so the device conv is
    one dense matmul.
  - Semantic gate fast path: cls and reg_pc are gated by
    sigmoid(sem) > 0.15, i.e. sem_logit + sem_b > logit(0.15).  The host
    checks the rigorous bound max_i ||feats_i|| * max_c ||sem_w_c|| +
    sem_b_c < logit(0.15); when it holds (it does for the detection-prior
    bias -4.595 used here), every mask entry is exactly zero, so cls and
    reg_pc are exactly zero and the device skips them.  If the bound ever
    fails, a full device path (mask + masked heads) is built instead.
  - Device fast path (identical SPMD program on 8 cores): inputs are
    SBUF-resident (a few large DMAs); work proceeds in 1024-column
    supertiles.  Three 128x128 bf16 matmul chains (mlp1, conv, mlp2) with
    ELU+1 done as one scalar Exp + one relu + one min (exact); the three
    skinny heads (sem 18, voff 3, cen 1) share one [65,T] PSUM tile via
    32-row tile_position strips and drain with a single scalar op.
    Outputs are bf16 and stream out per-supertile; the host re-transposes
    and fills the zero sections during unsharding.
"""

import numpy as np
import ml_dtypes

import concourse.bass as bass
import concourse.bacc as bacc
import concourse.tile as tile
from concourse import mybir
from concourse.bass_utils import run_bass_kernel_spmd

BF16 = ml_dtypes.bfloat16

N_VOX = 100000
C = 128
N_CLS = 18
N_REG = 6
VS = 0.04
THR = 0.15
HASH_D = 260
N_CORES = 8
PER_CORE = N_VOX // N_CORES          # 12500
ST = 1024                            # supertile columns
N_ST = 13                            # 12 full + tail of 212
LOGIT_THR = float(np.log(THR / (1.0 - THR)))   # -1.734601..

FOH = 127                            # full-path head rows [cls|cen|regJ]
OUT_ROWS = 151

F32 = mybir.dt.float32
BF = mybir.dt.bfloat16
AOp = mybir.AluOpType
Act = mybir.ActivationFunctionType


def _build_fast():
    """Fast path: masks provably all-zero; compute sem/voff/voted/cen only."""
    nc = bacc.Bacc(trn_type="TRN2")

    xT_d = nc.dram_tensor("xT", [C, PER_CORE], BF, kind="ExternalInput")
    gT_d = nc.dram_tensor("gT", [C, PER_CORE], BF, kind="ExternalInput")
    cvs_d = nc.dram_tensor("cvs", [3, PER_CORE], BF, kind="ExternalInput")
    # bf16 weights: w1 0:128, wc 128:256, w2 256:384, sem 384:402,
    # w3 402:405, cen 405:406
    wb_d = nc.dram_tensor("wb", [C, 406], BF, kind="ExternalInput")
    # f32 per-partition scalars [128, 11]: col0 b1, col1 bc, col2 b2,
    # col3 bias65 (rows 0:18 semb, 32:35 c3, 64 cenb),
    # col4 mn (rows 32:35), col5 mx (rows 32:35), col6-8 b1/bc/b2 + 1
    sc_d = nc.dram_tensor("sc", [C, 11], F32, kind="ExternalInput")
    outH_d = nc.dram_tensor("outH", [65, PER_CORE], BF, kind="ExternalOutput")
    outW_d = nc.dram_tensor("outW", [3, PER_CORE], BF, kind="ExternalOutput")

    chunks = [(0, 1024), (1024, 3072), (3072, 6144), (6144, 9216),
              (9216, PER_CORE)]

    with tile.TileContext(nc) as tc:
        with (
            tc.tile_pool(name="res", bufs=1) as res,
            tc.tile_pool(name="epool", bufs=4) as epool,
            tc.tile_pool(name="rpool", bufs=4) as rpool,
            tc.tile_pool(name="fpool", bufs=4) as fpool,
            tc.tile_pool(name="hpool", bufs=3) as hpool,
            tc.tile_pool(name="pbig", bufs=3, space=bass.MemorySpace.PSUM) as pbig,
            tc.tile_pool(name="phead", bufs=1, space=bass.MemorySpace.PSUM) as ph,
        ):
            wb = res.tile([C, 406], BF)
            sc = res.tile([C, 11], F32)
            nc.sync.dma_start(wb[:], wb_d[:])
            nc.sync.dma_start(sc[:], sc_d[:])
            w1 = wb[:, 0:128]
            wc = wb[:, 128:256]
            w2 = wb[:, 256:384]
            semw = wb[:, 384:402]
            w3 = wb[:, 402:405]
            cenw = wb[:, 405:406]
            b1 = sc[:, 0:1]
            bc = sc[:, 1:2]
            b2 = sc[:, 2:3]
            bias65 = sc[0:65, 3:4]
            mn3 = sc[32:35, 4:5]
            mx3 = sc[32:35, 5:6]
            b1p1 = sc[:, 6:7]
            bcp1 = sc[:, 7:8]
            b2p1 = sc[:, 8:9]

            xT = res.tile([C, PER_CORE], BF)
            gT = res.tile([C, PER_CORE], BF)
            cvsr = res.tile([35, PER_CORE], BF)
            for a, b in chunks:
                nc.sync.dma_start(xT[:, a:b], xT_d[:, a:b])
                nc.sync.dma_start(gT[:, a:b], gT_d[:, a:b])
                nc.sync.dma_start(cvsr[32:35, a:b], cvs_d[:, a:b])

            def mm_blocks(p, w, src_sb, off, L, rows=None, tp=None):
                for a in range(0, L, 512):
                    e = min(a + 512, L)
                    dst = p[:, a:e] if rows is None else p[rows[0]:rows[1], a:e]
                    if tp is None:
                        nc.tensor.matmul(dst, w, src_sb[:, off + a:off + e],
                                         start=True, stop=True)
                    else:
                        nc.tensor.matmul(dst, w, src_sb[:, off + a:off + e],
                                         start=True, stop=True,
                                         tile_position=tp)

            def elu_tail(p, bias, biasp1, L, scalar_relu=False):
                # f = min(max(y+b+1, 1), exp(y+b)) == ELU(y+b) + 1, exact
                et = epool.tile([C, ST], BF, tag="e")
                nc.scalar.activation(et[:, :L], p[:, :L], Act.Exp, bias=bias)
                ft = fpool.tile([C, ST], BF, tag="f")
                if scalar_relu:
                    # relu on Scalar, (r+1) min e on Vector (STT)
                    rt = rpool.tile([C, ST], BF, tag="r")
                    nc.scalar.activation(rt[:, :L], p[:, :L], Act.Relu,
                                         bias=bias)
                    nc.vector.scalar_tensor_tensor(
                        ft[:, :L], rt[:, :L], 1.0, et[:, :L],
                        AOp.add, AOp.min)
                else:
                    # max(y+b+1, 1) on Vector (TS), min on Vector (TT, 2x)
                    rt = rpool.tile([C, ST], BF, tag="r")
                    nc.vector.tensor_scalar(rt[:, :L], p[:, :L], biasp1, 1.0,
                                            AOp.add, AOp.max)
                    nc.vector.tensor_tensor(ft[:, :L], rt[:, :L], et[:, :L],
                                            AOp.min)
                return ft

            def head_tail(st):
                # finish supertile st: all three skinny head matmuls grouped
                # back-to-back (deps long ready; disjoint col strips), then
                # drain, voted chain, stores
                cs0, L0, f20, fo0 = st
                phd0 = ph.tile([65, ST], F32, tag="ph")
                for a in range(0, L0, 512):
                    e = min(a + 512, L0)
                    nc.tensor.matmul(phd0[0:18, a:e], semw,
                                     xT[:, cs0 + a:cs0 + e],
                                     start=True, stop=True,
                                     tile_position=(0, 0))
                    nc.tensor.matmul(phd0[32:35, a:e], w3, f20[:, a:e],
                                     start=True, stop=True,
                                     tile_position=(0, 32))
                    nc.tensor.matmul(phd0[64:65, a:e], cenw, fo0[:, a:e],
                                     start=True, stop=True,
                                     tile_position=(0, 64))
                sv = hpool.tile([65, ST], BF, tag="sv")
                nc.scalar.activation(sv[:, :L0], phd0[:, :L0], Act.Identity,
                                     bias=bias65)
                nc.sync.dma_start(outH_d[:, cs0:cs0 + L0], sv[:, :L0])
                vp = hpool.tile([35, ST], BF, tag="vp")
                nc.gpsimd.tensor_tensor(vp[32:35, :L0], sv[32:35, :L0],
                                        cvsr[32:35, cs0:cs0 + L0], AOp.add)
                vt = hpool.tile([35, ST], BF, tag="vt")
                nc.vector.tensor_scalar(vt[32:35, :L0], vp[32:35, :L0],
                                        mn3, mx3, AOp.max, AOp.min)
                nc.sync.dma_start(outW_d[:, cs0:cs0 + L0], vt[32:35, :L0])

            pending = None
            for k in range(N_ST):
                cs = k * ST
                L = min(ST, PER_CORE - cs)

                # mlp1 + conv matmuls, then sem (xT-only) early for overlap
                p1 = pbig.tile([C, ST], F32, tag="p")
                mm_blocks(p1, w1, xT, cs, L)
                pc = pbig.tile([C, ST], F32, tag="p")
                mm_blocks(pc, wc, gT, cs, L)
                if pending is not None:
                    head_tail(pending)

                f1 = elu_tail(p1, b1, b1p1, L)
                fo = elu_tail(pc, bc, bcp1, L, scalar_relu=True)

                p2 = pbig.tile([C, ST], F32, tag="p")
                mm_blocks(p2, w2, f1, 0, L)
                f2 = elu_tail(p2, b2, b2p1, L)
                pending = (cs, L, f2, fo)

            head_tail(pending)

    nc.finalize()
    return nc


def _build_full():
    """Fallback: full mask + masked-heads path (used if the zero-mask
    bound fails).  j-major reg layout, one is_gt + one STT for heads."""
    nc = bacc.Bacc(trn_type="TRN2")

    xT_d = nc.dram_tensor("xT", [C, PER_CORE], BF, kind="ExternalInput")
    gT_d = nc.dram_tensor("gT", [C, PER_CORE], BF, kind="ExternalInput")
    cvs_d = nc.dram_tensor("cvs", [3, PER_CORE], BF, kind="ExternalInput")
    # w1 0:128, wc 128:256, w2 256:384, semJF 384:511, fohF 511:638,
    # w3 638:641, sem 641:659, cen 659:660
    wb_d = nc.dram_tensor("wb", [C, 660], BF, kind="ExternalInput")
    # col0 b2, col1 bias65, col2 mn, col3 mx, col4 thrF, col5 biasF
    sc_d = nc.dram_tensor("sc", [C, 8], F32, kind="ExternalInput")
    outH_d = nc.dram_tensor("outH", [65, PER_CORE], BF, kind="ExternalOutput")
    outW_d = nc.dram_tensor("outW", [3, PER_CORE], BF, kind="ExternalOutput")
    outB_d = nc.dram_tensor("outB", [FOH, PER_CORE], BF, kind="ExternalOutput")

    chunks = [(0, 3072), (3072, 6144), (6144, 9216), (9216, PER_CORE)]

    with tile.TileContext(nc) as tc:
        with (
            tc.tile_pool(name="res", bufs=1) as res,
            tc.tile_pool(name="epool", bufs=4) as epool,
            tc.tile_pool(name="rpool", bufs=4) as rpool,
            tc.tile_pool(name="fpool", bufs=4) as fpool,
            tc.tile_pool(name="hpool", bufs=3) as hpool,
            tc.tile_pool(name="mpool", bufs=2) as mpool,
            tc.tile_pool(name="pbig", bufs=2, space=bass.MemorySpace.PSUM) as pbig,
            tc.tile_pool(name="phead", bufs=1, space=bass.MemorySpace.PSUM) as ph,
            tc.tile_pool(name="pfoh", bufs=1, space=bass.MemorySpace.PSUM) as pf,
        ):
            wb = res.tile([C, 660], BF)
            sc = res.tile([C, 11], F32)
            nc.sync.dma_start(wb[:], wb_d[:])
            nc.sync.dma_start(sc[:], sc_d[:])
            w1 = wb[:, 0:128]
            wc = wb[:, 128:256]
            w2 = wb[:, 256:384]
            semJF = wb[:, 384:511]
            fohF = wb[:, 511:638]
            w3 = wb[:, 638:641]
            semw = wb[:, 641:659]
            cenw = wb[:, 659:660]
            b1 = sc[:, 0:1]
            bc = sc[:, 1:2]
            b2 = sc[:, 2:3]
            bias65 = sc[0:65, 3:4]
            mn3 = sc[32:35, 4:5]
            mx3 = sc[32:35, 5:6]
            b1p1 = sc[:, 6:7]
            bcp1 = sc[:, 7:8]
            b2p1 = sc[:, 8:9]
            thrF = sc[0:FOH, 9:10]
            biasF = sc[0:FOH, 10:11]

            xT = res.tile([C, PER_CORE], BF)
            gT = res.tile([C, PER_CORE], BF)
            cvsr = res.tile([35, PER_CORE], BF)
            for a, b in chunks:
                nc.sync.dma_start(xT[:, a:b], xT_d[:, a:b])
                nc.sync.dma_start(gT[:, a:b], gT_d[:, a:b])
                nc.sync.dma_start(cvsr[32:35, a:b], cvs_d[:, a:b])

            for k in range(N_ST):
                cs = k * ST
                L = min(ST, PER_CORE - cs)

                def elu_layer(w, src_sb, off, bias):
                    p = pbig.tile([C, ST], F32, tag="p")
                    for a in range(0, L, 512):
                        e = min(a + 512, L)
                        nc.tensor.matmul(p[:, a:e], w,
                                         src_sb[:, off + a:off + e],
                                         start=True, stop=True)
                    et = epool.tile([C, ST], BF, tag="e")
                    nc.scalar.activation(et[:, :L], p[:, :L], Act.Exp,
                                         bias=bias)
                    rt = rpool.tile([C, ST], BF, tag="r")
                    nc.vector.tensor_scalar(rt[:, :L], p[:, :L], bias,
                                            0.0, AOp.add, AOp.max)
                    ft = fpool.tile([C, ST], BF, tag="f")
                    nc.vector.scalar_tensor_tensor(
                        ft[:, :L], rt[:, :L], 1.0, et[:, :L],
                        AOp.add, AOp.min)
                    return ft

                f1 = elu_layer(w1, xT, cs, b1)
                fo = elu_layer(wc, gT, cs, bc)
                f2 = elu_layer(w2, f1, 0, b2)

                phd = ph.tile([65, ST], F32, tag="ph")
                for a in range(0, L, 512):
                    e = min(a + 512, L)
                    nc.tensor.matmul(phd[0:18, a:e], semw,
                                     xT[:, cs + a:cs + e],
                                     start=True, stop=True,
                                     tile_position=(0, 0))
                    nc.tensor.matmul(phd[32:35, a:e], w3, f2[:, a:e],
                                     start=True, stop=True,
                                     tile_position=(0, 32))
                    nc.tensor.matmul(phd[64:65, a:e], cenw, fo[:, a:e],
                                     start=True, stop=True,
                                     tile_position=(0, 64))
                sv = hpool.tile([65, ST], BF, tag="sv")
                nc.scalar.activation(sv[:, :L], phd[:, :L], Act.Identity,
                                     bias=bias65)
                nc.sync.dma_start(outH_d[:, cs:cs + L], sv[:, :L])

                vp = hpool.tile([35, ST], BF, tag="vp")
                nc.gpsimd.tensor_tensor(vp[32:35, :L], sv[32:35, :L],
                                        cvsr[32:35, cs:cs + L], AOp.add)
                vt = hpool.tile([35, ST], BF, tag="vt")
                nc.vector.tensor_scalar(vt[32:35, :L], vp[32:35, :L],
                                        mn3, mx3, AOp.max, AOp.min)
                nc.sync.dma_start(outW_d[:, cs:cs + L], vt[32:35, :L])

                # masked heads: (foh + biasF) * (semJ > thrF)
                psem = ph.tile([FOH, ST], F32, tag="ph")
                for a in range(0, L, 512):
                    e = min(a + 512, L)
                    nc.tensor.matmul(psem[:, a:e], semJF,
                                     xT[:, cs + a:cs + e],
                                     start=True, stop=True)
                maskF = mpool.tile([FOH, ST], BF, tag="maskF")
                nc.vector.tensor_scalar(maskF[:, :L], psem[:, :L], thrF,
                                        None, AOp.is_gt)
                pfo = pf.tile([FOH, ST], F32, tag="pfoh")
                for a in range(0, L, 512):
                    e = min(a + 512, L)
                    nc.tensor.matmul(pfo[:, a:e], fohF, fo[:, a:e],
                                     start=True, stop=True)
                hb = mpool.tile([FOH, ST], BF, tag="hb")
                nc.vector.scalar_tensor_tensor(
                    hb[:, :L], pfo[:, :L], biasF, maskF[:, :L],
                    AOp.add, AOp.mult)
                nc.sync.dma_start(outB_d[:, cs:cs + L], hb[:, :L])

    nc.finalize()
    return nc


def _host_prep(feats, coords_xyz, batch_idx,
               off_w1, off_g1, off_b1, off_w2, off_g2, off_b2, off_w3,
               fo_w, fo_g, fo_b, sem_w, sem_b, cen_w, cls_w, cls_b, reg_w,
               scales):
    f64 = np.float64
    N = feats.shape[0]

    # ---- zero-mask bound: |x.w_c| <= max||x|| * ||w_c|| (Cauchy-Schwarz)
    feats64 = feats.astype(f64)
    max_row = np.sqrt((feats64 * feats64).sum(1).max())
    colnrm = np.sqrt((sem_w.astype(f64) ** 2).sum(0))
    zero_mask = bool(
        np.all(max_row * colnrm + sem_b.astype(f64) < LOGIT_THR - 1e-3))

    # ---- neighbor lookup (identical to reference's sorted-key search) ----
    c1 = coords_xyz.astype(np.int64) + 1
    key = ((batch_idx.astype(np.int64) * HASH_D + c1[:, 0]) * HASH_D
           + c1[:, 1]) * HASH_D + c1[:, 2]
    order = np.argsort(key, kind="stable")
    skey = key[order]
    pos = np.searchsorted(skey, key)
    rep = order[pos]                      # first voxel with same key

    # ---- fused weights (BN folded; ELU+1 handled via bias shifts) ----
    W1 = off_w1.astype(f64) * off_g1.astype(f64)[None, :]
    b1 = off_b1.astype(f64)
    W2 = off_w2.astype(f64) * off_g2.astype(f64)[None, :]
    b2 = off_b2.astype(f64) - W2.sum(0)
    W3 = off_w3.astype(f64)
    c3 = -W3.sum(0)
    Wc = fo_w[13].astype(f64) * fo_g.astype(f64)[None, :]
    bc = fo_b.astype(f64)

    # ---- conv input: gather + fold rare non-center taps via Wc13^-1 ----
    G = feats64[rep]
    Winv = np.linalg.inv(fo_w[13].astype(f64))
    k = 0
    for dx in (-1, 0, 1):
        for dy in (-1, 0, 1):
            for dz in (-1, 0, 1):
                if (dx, dy, dz) != (0, 0, 0):
                    nk = key + (dx * HASH_D + dy) * HASH_D + dz
                    p = np.clip(np.searchsorted(skey, nk), 0, N - 1)
                    hit = skey[p] == nk
                    if hit.any():
                        dst = np.nonzero(hit)[0]
                        src = order[p[hit]]
                        A = fo_w[k].astype(f64) @ Winv
                        np.add.at(G, dst, feats64[src] @ A)
                k += 1

    # ---- shared scalar columns ----
    mx = (coords_xyz.max(0) + 1).astype(f64) * VS
    mn = (coords_xyz.min(0) - 1).astype(f64) * VS
    bias65 = np.zeros(65, f64)
    bias65[0:N_CLS] = sem_b.astype(f64)
    bias65[32:35] = c3
    bias65[64] = -cen_w.astype(f64).sum(0)[0]

    sc = np.zeros((C, 11), np.float32)
    sc[:, 0] = b1
    sc[:, 1] = bc
    sc[:, 2] = b2
    sc[0:65, 3] = bias65
    sc[32:35, 4] = mn
    sc[32:35, 5] = mx
    sc[:, 6] = b1 + 1.0
    sc[:, 7] = bc + 1.0
    sc[:, 8] = b2 + 1.0

    nwb = 406 if zero_mask else 660
    wb = np.zeros((C, nwb), BF16)
    wb[:, 0:128] = W1.astype(BF16)
    wb[:, 128:256] = Wc.astype(BF16)
    wb[:, 256:384] = W2.astype(BF16)
    if zero_mask:
        wb[:, 384:402] = sem_w.astype(f64).astype(BF16)
        wb[:, 402:405] = W3.astype(BF16)
        wb[:, 405:406] = cen_w.astype(f64).astype(BF16)
    else:
        # j-major layout: rows [cls 0:18 | cen 18 | regJ 19:127]
        sc64 = scales.astype(f64)
        semJF = np.zeros((C, FOH), f64)
        fohF = np.zeros((C, FOH), f64)
        thrF = np.zeros(FOH, np.float32)
        biasF = np.zeros(FOH, np.float32)
        semJF[:, 0:N_CLS] = sem_w.astype(f64)
        fohF[:, 0:N_CLS] = cls_w.astype(f64)
        thrF[0:N_CLS] = LOGIT_THR - sem_b.astype(f64)
        biasF[0:N_CLS] = cls_b.astype(f64) - cls_w.astype(f64).sum(0)
        fohF[:, N_CLS] = cen_w.astype(f64)[:, 0]
        thrF[N_CLS] = -1e30
        biasF[N_CLS] = -cen_w.astype(f64).sum(0)[0]
        for j in range(N_REG):
            for cc in range(N_CLS):
                r = 19 + j * N_CLS + cc
                semJF[:, r] = sem_w.astype(f64)[:, cc]
                fohF[:, r] = reg_w.astype(f64)[:, j] * sc64[cc]
                thrF[r] = LOGIT_THR - sem_b.astype(f64)[cc]
                biasF[r] = -reg_w.astype(f64)[:, j].sum() * sc64[cc]
        wb[:, 384:511] = semJF.astype(BF16)
        wb[:, 511:638] = fohF.astype(BF16)
        wb[:, 638:641] = W3.astype(BF16)
        wb[:, 641:659] = sem_w.astype(f64).astype(BF16)
        wb[:, 659:660] = cen_w.astype(f64).astype(BF16)
        sc[0:FOH, 9] = thrF
        sc[0:FOH, 10] = biasF

    # ---- transposed, channel-major activations ----
    fT = np.ascontiguousarray(feats.T)
    gTf = np.ascontiguousarray(G.astype(np.float32).T)
    cT = coords_xyz.T.astype(np.float32) * VS

    wts = {"wb": wb, "sc": sc}
    in_maps = []
    for c in range(N_CORES):
        s, e = c * PER_CORE, (c + 1) * PER_CORE
        m = dict(wts)
        m["xT"] = np.ascontiguousarray(fT[:, s:e]).astype(BF16)
        m["gT"] = np.ascontiguousarray(gTf[:, s:e]).astype(BF16)
        m["cvs"] = np.ascontiguousarray(cT[:, s:e]).astype(BF16)
        in_maps.append(m)
    return in_maps, zero_mask


_CACHED = {}

# full path: regJ row 19 + j*18 + c  ->  output col 43 + c*6 + j
_REG_COLS = np.empty(FOH - 19, np.int64)
for _j in range(N_REG):
    for _c in range(N_CLS):
        _REG_COLS[_j * N_CLS + _c] = 43 + _c * N_REG + _j


def _assemble(res, zero_mask, n):
    """Map device outputs to reference layout [n, 151] f32."""
    o = np.zeros((n, OUT_ROWS), np.float32)
    outH = res["outH"][:, :n].astype(np.float32)
    o[:, 0:18] = outH[0:18].T
    o[:, 18:21] = outH[32:35].T
    o[:, 21:24] = res["outW"][:, :n].astype(np.float32).T
    o[:, 24] = outH[64]
    if not zero_mask:
        outB = res["outB"][:, :n].astype(np.float32)
        o[:, 25:43] = outB[0:N_CLS].T
        o[:, _REG_COLS] = outB[19:FOH].T
    return o


def kernel(**inputs):
    inputs = {k: np.asarray(v) for k, v in inputs.items()}
    in_maps, zero_mask = _host_prep(**inputs)
    ckey = "fast" if zero_mask else "full"
    if ckey not in _CACHED:
        _CACHED[ckey] = _build_fast() if zero_mask else _build_full()
    nc = _CACHED[ckey]
    res = run_bass_kernel_spmd(nc, in_maps, core_ids=list(range(N_CORES)))
    out = np.empty((N_VOX, OUT_ROWS), np.float32)
    for c in range(N_CORES):
        out[c * PER_CORE:(c + 1) * PER_CORE] = _assemble(
            res.results[c], zero_mask, PER_CORE)
    return out


# revision 14
# speedup vs baseline: 2.1339x; 1.0041x over previous
"""CAGroup3DHead kernel for 8 Trainium2 NeuronCores.

Strategy (data-parallel over voxels, per the sharding hint):
  - Host: integer index work (sorted-key neighbor lookup identical to the
    reference), weight fusion (BN folded into weights, ELU+1 bias shifts),
    and sharding marshaling (transpose to channel-major, bf16 cast,
    per-core slices).  The 3x3x3 sparse conv collapses to a gather: the
    (0,0,0) tap always hits, so conv_in = feats[rep]; the rare other-tap
    hits are folded into conv_in via W_k @ W_13^{-1} so the device conv is
    one dense matmul.
  - Semantic gate fast path: cls and reg_pc are gated by
    sigmoid(sem) > 0.15, i.e. sem_logit + sem_b > logit(0.15).  The host
    checks the rigorous bound max_i ||feats_i|| * max_c ||sem_w_c|| +
    sem_b_c < logit(0.15); when it holds (it does for the detection-prior
    bias -4.595 used here), every mask entry is exactly zero, so cls and
    reg_pc are exactly zero and the device skips them.  If the bound ever
    fails, a full device path (mask + masked heads) is built instead.
  - Device fast path (identical SPMD program on 8 cores): inputs are
    SBUF-resident (a few large DMAs); work proceeds in 1024-column
    supertiles.  Three 128x128 bf16 matmul chains (mlp1, conv, mlp2) with
    ELU+1 done as one scalar Exp + one relu + one min (exact); the three
    skinny heads (sem 18, voff 3, cen 1) share one [65,T] PSUM tile via
    32-row tile_position strips and drain with a single scalar op.
    Outputs are bf16 and stream out per-supertile; the host re-transposes
    and fills the zero sections during unsharding.
"""

import numpy as np
import ml_dtypes

import concourse.bass as bass
import concourse.bacc as bacc
import concourse.tile as tile
from concourse import mybir
from concourse.bass_utils import run_bass_kernel_spmd

BF16 = ml_dtypes.bfloat16

N_VOX = 100000
C = 128
N_CLS = 18
N_REG = 6
VS = 0.04
THR = 0.15
HASH_D = 260
N_CORES = 8
PER_CORE = N_VOX // N_CORES          # 12500
ST = 1024                            # supertile columns
N_ST = 13                            # 12 full + tail of 212
LOGIT_THR = float(np.log(THR / (1.0 - THR)))   # -1.734601..

FOH = 127                            # full-path head rows [cls|cen|regJ]
OUT_ROWS = 151

F32 = mybir.dt.float32
BF = mybir.dt.bfloat16
AOp = mybir.AluOpType
Act = mybir.ActivationFunctionType


def _build_fast():
    """Fast path: masks provably all-zero; compute sem/voff/voted/cen only."""
    nc = bacc.Bacc(trn_type="TRN2")

    xT_d = nc.dram_tensor("xT", [C, PER_CORE], BF, kind="ExternalInput")
    gT_d = nc.dram_tensor("gT", [C, PER_CORE], BF, kind="ExternalInput")
    cvs_d = nc.dram_tensor("cvs", [3, PER_CORE], BF, kind="ExternalInput")
    # bf16 weights: w1 0:128, wc 128:256, w2 256:384, sem 384:402,
    # w3 402:405, cen 405:406
    wb_d = nc.dram_tensor("wb", [C, 406], BF, kind="ExternalInput")
    # f32 per-partition scalars [128, 11]: col0 b1, col1 bc, col2 b2,
    # col3 bias65 (rows 0:18 semb, 32:35 c3, 64 cenb),
    # col4 mn (rows 32:35), col5 mx (rows 32:35), col6-8 b1/bc/b2 + 1
    sc_d = nc.dram_tensor("sc", [C, 11], F32, kind="ExternalInput")
    outH_d = nc.dram_tensor("outH", [65, PER_CORE], BF, kind="ExternalOutput")
    outW_d = nc.dram_tensor("outW", [3, PER_CORE], BF, kind="ExternalOutput")

    chunks = [(0, 1024), (1024, 3072), (3072, 6144), (6144, 9216),
              (9216, PER_CORE)]

    with tile.TileContext(nc) as tc:
        with (
            tc.tile_pool(name="res", bufs=1) as res,
            tc.tile_pool(name="epool", bufs=4) as epool,
            tc.tile_pool(name="rpool", bufs=4) as rpool,
            tc.tile_pool(name="fpool", bufs=4) as fpool,
            tc.tile_pool(name="hpool", bufs=3) as hpool,
            tc.tile_pool(name="pbig", bufs=3, space=bass.MemorySpace.PSUM) as pbig,
            tc.tile_pool(name="phead", bufs=1, space=bass.MemorySpace.PSUM) as ph,
        ):
            wb = res.tile([C, 406], BF)
            sc = res.tile([C, 11], F32)
            nc.sync.dma_start(wb[:], wb_d[:])
            nc.sync.dma_start(sc[:], sc_d[:])
            w1 = wb[:, 0:128]
            wc = wb[:, 128:256]
            w2 = wb[:, 256:384]
            semw = wb[:, 384:402]
            w3 = wb[:, 402:405]
            cenw = wb[:, 405:406]
            b1 = sc[:, 0:1]
            bc = sc[:, 1:2]
            b2 = sc[:, 2:3]
            bias65 = sc[0:65, 3:4]
            mn3 = sc[32:35, 4:5]
            mx3 = sc[32:35, 5:6]
            b1p1 = sc[:, 6:7]
            bcp1 = sc[:, 7:8]
            b2p1 = sc[:, 8:9]

            xT = res.tile([C, PER_CORE], BF)
            gT = res.tile([C, PER_CORE], BF)
            cvsr = res.tile([35, PER_CORE], BF)
            for a, b in chunks:
                nc.sync.dma_start(xT[:, a:b], xT_d[:, a:b])
                nc.sync.dma_start(gT[:, a:b], gT_d[:, a:b])
                nc.sync.dma_start(cvsr[32:35, a:b], cvs_d[:, a:b])

            def mm_blocks(p, w, src_sb, off, L, rows=None, tp=None):
                for a in range(0, L, 512):
                    e = min(a + 512, L)
                    dst = p[:, a:e] if rows is None else p[rows[0]:rows[1], a:e]
                    if tp is None:
                        nc.tensor.matmul(dst, w, src_sb[:, off + a:off + e],
                                         start=True, stop=True)
                    else:
                        nc.tensor.matmul(dst, w, src_sb[:, off + a:off + e],
                                         start=True, stop=True,
                                         tile_position=tp)

            def elu_tail(p, bias, biasp1, L, scalar_relu=False):
                # f = min(max(y+b+1, 1), exp(y+b)) == ELU(y+b) + 1, exact
                et = epool.tile([C, ST], BF, tag="e")
                nc.scalar.activation(et[:, :L], p[:, :L], Act.Exp, bias=bias)
                ft = fpool.tile([C, ST], BF, tag="f")
                if scalar_relu:
                    # relu on Scalar, (r+1) min e on Vector (STT)
                    rt = rpool.tile([C, ST], BF, tag="r")
                    nc.scalar.activation(rt[:, :L], p[:, :L], Act.Relu,
                                         bias=bias)
                    nc.vector.scalar_tensor_tensor(
                        ft[:, :L], rt[:, :L], 1.0, et[:, :L],
                        AOp.add, AOp.min)
                else:
                    # max(y+b+1, 1) on Vector (TS), min on Vector (TT, 2x)
                    rt = rpool.tile([C, ST], BF, tag="r")
                    nc.vector.tensor_scalar(rt[:, :L], p[:, :L], biasp1, 1.0,
                                            AOp.add, AOp.max)
                    nc.vector.tensor_tensor(ft[:, :L], rt[:, :L], et[:, :L],
                                            AOp.min)
                return ft

            def head_tail(st):
                # finish supertile st: all three skinny head matmuls grouped
                # back-to-back (deps long ready; disjoint col strips), then
                # drain, voted chain, stores
                cs0, L0, f20, fo0 = st
                phd0 = ph.tile([65, ST], F32, tag="ph")
                for a in range(0, L0, 512):
                    e = min(a + 512, L0)
                    nc.tensor.matmul(phd0[0:18, a:e], semw,
                                     xT[:, cs0 + a:cs0 + e],
                                     start=True, stop=True,
                                     tile_position=(0, 0))
                    nc.tensor.matmul(phd0[32:35, a:e], w3, f20[:, a:e],
                                     start=True, stop=True,
                                     tile_position=(0, 32))
                    nc.tensor.matmul(phd0[64:65, a:e], cenw, fo0[:, a:e],
                                     start=True, stop=True,
                                     tile_position=(0, 64))
                sv = hpool.tile([65, ST], BF, tag="sv")
                nc.scalar.activation(sv[:, :L0], phd0[:, :L0], Act.Identity,
                                     bias=bias65)
                nc.sync.dma_start(outH_d[:, cs0:cs0 + L0], sv[:, :L0])
                vp = hpool.tile([35, ST], BF, tag="vp")
                nc.gpsimd.tensor_tensor(vp[32:35, :L0], sv[32:35, :L0],
                                        cvsr[32:35, cs0:cs0 + L0], AOp.add)
                vt = hpool.tile([35, ST], BF, tag="vt")
                nc.vector.tensor_scalar(vt[32:35, :L0], vp[32:35, :L0],
                                        mn3, mx3, AOp.max, AOp.min)
                nc.sync.dma_start(outW_d[:, cs0:cs0 + L0], vt[32:35, :L0])

            pending = None
            for k in range(N_ST):
                cs = k * ST
                L = min(ST, PER_CORE - cs)

                # mlp1 + conv matmuls, then sem (xT-only) early for overlap
                p1 = pbig.tile([C, ST], F32, tag="p")
                mm_blocks(p1, w1, xT, cs, L)
                pc = pbig.tile([C, ST], F32, tag="p")
                mm_blocks(pc, wc, gT, cs, L)
                if pending is not None:
                    head_tail(pending)

                f1 = elu_tail(p1, b1, b1p1, L)
                fo = elu_tail(pc, bc, bcp1, L, scalar_relu=True)

                p2 = pbig.tile([C, ST], F32, tag="p")
                mm_blocks(p2, w2, f1, 0, L)
                f2 = elu_tail(p2, b2, b2p1, L)
                pending = (cs, L, f2, fo)

            head_tail(pending)

    nc.finalize()
    return nc


def _build_full():
    """Fallback: full mask + masked-heads path (used if the zero-mask
    bound fails).  j-major reg layout, one is_gt + one STT for heads."""
    nc = bacc.Bacc(trn_type="TRN2")

    xT_d = nc.dram_tensor("xT", [C, PER_CORE], BF, kind="ExternalInput")
    gT_d = nc.dram_tensor("gT", [C, PER_CORE], BF, kind="ExternalInput")
    cvs_d = nc.dram_tensor("cvs", [3, PER_CORE], BF, kind="ExternalInput")
    # w1 0:128, wc 128:256, w2 256:384, semJF 384:511, fohF 511:638,
    # w3 638:641, sem 641:659, cen 659:660
    wb_d = nc.dram_tensor("wb", [C, 660], BF, kind="ExternalInput")
    # col0 b2, col1 bias65, col2 mn, col3 mx, col4 thrF, col5 biasF
    sc_d = nc.dram_tensor("sc", [C, 8], F32, kind="ExternalInput")
    outH_d = nc.dram_tensor("outH", [65, PER_CORE], BF, kind="ExternalOutput")
    outW_d = nc.dram_tensor("outW", [3, PER_CORE], BF, kind="ExternalOutput")
    outB_d = nc.dram_tensor("outB", [FOH, PER_CORE], BF, kind="ExternalOutput")

    chunks = [(0, 3072), (3072, 6144), (6144, 9216), (9216, PER_CORE)]

    with tile.TileContext(nc) as tc:
        with (
            tc.tile_pool(name="res", bufs=1) as res,
            tc.tile_pool(name="epool", bufs=4) as epool,
            tc.tile_pool(name="rpool", bufs=4) as rpool,
            tc.tile_pool(name="fpool", bufs=4) as fpool,
            tc.tile_pool(name="hpool", bufs=3) as hpool,
            tc.tile_pool(name="mpool", bufs=2) as mpool,
            tc.tile_pool(name="pbig", bufs=2, space=bass.MemorySpace.PSUM) as pbig,
            tc.tile_pool(name="phead", bufs=1, space=bass.MemorySpace.PSUM) as ph,
            tc.tile_pool(name="pfoh", bufs=1, space=bass.MemorySpace.PSUM) as pf,
        ):
            wb = res.tile([C, 660], BF)
            sc = res.tile([C, 11], F32)
            nc.sync.dma_start(wb[:], wb_d[:])
            nc.sync.dma_start(sc[:], sc_d[:])
            w1 = wb[:, 0:128]
            wc = wb[:, 128:256]
            w2 = wb[:, 256:384]
            semJF = wb[:, 384:511]
            fohF = wb[:, 511:638]
            w3 = wb[:, 638:641]
            semw = wb[:, 641:659]
            cenw = wb[:, 659:660]
            b1 = sc[:, 0:1]
            bc = sc[:, 1:2]
            b2 = sc[:, 2:3]
            bias65 = sc[0:65, 3:4]
            mn3 = sc[32:35, 4:5]
            mx3 = sc[32:35, 5:6]
            b1p1 = sc[:, 6:7]
            bcp1 = sc[:, 7:8]
            b2p1 = sc[:, 8:9]
            thrF = sc[0:FOH, 9:10]
            biasF = sc[0:FOH, 10:11]

            xT = res.tile([C, PER_CORE], BF)
            gT = res.tile([C, PER_CORE], BF)
            cvsr = res.tile([35, PER_CORE], BF)
            for a, b in chunks:
                nc.sync.dma_start(xT[:, a:b], xT_d[:, a:b])
                nc.sync.dma_start(gT[:, a:b], gT_d[:, a:b])
                nc.sync.dma_start(cvsr[32:35, a:b], cvs_d[:, a:b])

            for k in range(N_ST):
                cs = k * ST
                L = min(ST, PER_CORE - cs)

                def elu_layer(w, src_sb, off, bias):
                    p = pbig.tile([C, ST], F32, tag="p")
                    for a in range(0, L, 512):
                        e = min(a + 512, L)
                        nc.tensor.matmul(p[:, a:e], w,
                                         src_sb[:, off + a:off + e],
                                         start=True, stop=True)
                    et = epool.tile([C, ST], BF, tag="e")
                    nc.scalar.activation(et[:, :L], p[:, :L], Act.Exp,
                                         bias=bias)
                    rt = rpool.tile([C, ST], BF, tag="r")
                    nc.vector.tensor_scalar(rt[:, :L], p[:, :L], bias,
                                            0.0, AOp.add, AOp.max)
                    ft = fpool.tile([C, ST], BF, tag="f")
                    nc.vector.scalar_tensor_tensor(
                        ft[:, :L], rt[:, :L], 1.0, et[:, :L],
                        AOp.add, AOp.min)
                    return ft

                f1 = elu_layer(w1, xT, cs, b1)
                fo = elu_layer(wc, gT, cs, bc)
                f2 = elu_layer(w2, f1, 0, b2)

                phd = ph.tile([65, ST], F32, tag="ph")
                for a in range(0, L, 512):
                    e = min(a + 512, L)
                    nc.tensor.matmul(phd[0:18, a:e], semw,
                                     xT[:, cs + a:cs + e],
                                     start=True, stop=True,
                                     tile_position=(0, 0))
                    nc.tensor.matmul(phd[32:35, a:e], w3, f2[:, a:e],
                                     start=True, stop=True,
                                     tile_position=(0, 32))
                    nc.tensor.matmul(phd[64:65, a:e], cenw, fo[:, a:e],
                                     start=True, stop=True,
                                     tile_position=(0, 64))
                sv = hpool.tile([65, ST], BF, tag="sv")
                nc.scalar.activation(sv[:, :L], phd[:, :L], Act.Identity,
                                     bias=bias65)
                nc.sync.dma_start(outH_d[:, cs:cs + L], sv[:, :L])

                vp = hpool.tile([35, ST], BF, tag="vp")
                nc.gpsimd.tensor_tensor(vp[32:35, :L], sv[32:35, :L],
                                        cvsr[32:35, cs:cs + L], AOp.add)
                vt = hpool.tile([35, ST], BF, tag="vt")
                nc.vector.tensor_scalar(vt[32:35, :L], vp[32:35, :L],
                                        mn3, mx3, AOp.max, AOp.min)
                nc.sync.dma_start(outW_d[:, cs:cs + L], vt[32:35, :L])

                # masked heads: (foh + biasF) * (semJ > thrF)
                psem = ph.tile([FOH, ST], F32, tag="ph")
                for a in range(0, L, 512):
                    e = min(a + 512, L)
                    nc.tensor.matmul(psem[:, a:e], semJF,
                                     xT[:, cs + a:cs + e],
                                     start=True, stop=True)
                maskF = mpool.tile([FOH, ST], BF, tag="maskF")
                nc.vector.tensor_scalar(maskF[:, :L], psem[:, :L], thrF,
                                        None, AOp.is_gt)
                pfo = pf.tile([FOH, ST], F32, tag="pfoh")
                for a in range(0, L, 512):
                    e = min(a + 512, L)
                    nc.tensor.matmul(pfo[:, a:e], fohF, fo[:, a:e],
                                     start=True, stop=True)
                hb = mpool.tile([FOH, ST], BF, tag="hb")
                nc.vector.scalar_tensor_tensor(
                    hb[:, :L], pfo[:, :L], biasF, maskF[:, :L],
                    AOp.add, AOp.mult)
                nc.sync.dma_start(outB_d[:, cs:cs + L], hb[:, :L])

    nc.finalize()
    return nc


def _host_prep(feats, coords_xyz, batch_idx,
               off_w1, off_g1, off_b1, off_w2, off_g2, off_b2, off_w3,
               fo_w, fo_g, fo_b, sem_w, sem_b, cen_w, cls_w, cls_b, reg_w,
               scales):
    f64 = np.float64
    N = feats.shape[0]

    # ---- zero-mask bound: |x.w_c| <= max||x|| * ||w_c|| (Cauchy-Schwarz)
    feats64 = feats.astype(f64)
    max_row = np.sqrt((feats64 * feats64).sum(1).max())
    colnrm = np.sqrt((sem_w.astype(f64) ** 2).sum(0))
    zero_mask = bool(
        np.all(max_row * colnrm + sem_b.astype(f64) < LOGIT_THR - 1e-3))

    # ---- neighbor lookup (identical to reference's sorted-key search) ----
    c1 = coords_xyz.astype(np.int64) + 1
    key = ((batch_idx.astype(np.int64) * HASH_D + c1[:, 0]) * HASH_D
           + c1[:, 1]) * HASH_D + c1[:, 2]
    order = np.argsort(key, kind="stable")
    skey = key[order]
    pos = np.searchsorted(skey, key)
    rep = order[pos]                      # first voxel with same key

    # ---- fused weights (BN folded; ELU+1 handled via bias shifts) ----
    W1 = off_w1.astype(f64) * off_g1.astype(f64)[None, :]
    b1 = off_b1.astype(f64)
    W2 = off_w2.astype(f64) * off_g2.astype(f64)[None, :]
    b2 = off_b2.astype(f64) - W2.sum(0)
    W3 = off_w3.astype(f64)
    c3 = -W3.sum(0)
    Wc = fo_w[13].astype(f64) * fo_g.astype(f64)[None, :]
    bc = fo_b.astype(f64)

    # ---- conv input: gather + fold rare non-center taps via Wc13^-1 ----
    G = feats64[rep]
    Winv = np.linalg.inv(fo_w[13].astype(f64))
    k = 0
    for dx in (-1, 0, 1):
        for dy in (-1, 0, 1):
            for dz in (-1, 0, 1):
                if (dx, dy, dz) != (0, 0, 0):
                    nk = key + (dx * HASH_D + dy) * HASH_D + dz
                    p = np.clip(np.searchsorted(skey, nk), 0, N - 1)
                    hit = skey[p] == nk
                    if hit.any():
                        dst = np.nonzero(hit)[0]
                        src = order[p[hit]]
                        A = fo_w[k].astype(f64) @ Winv
                        np.add.at(G, dst, feats64[src] @ A)
                k += 1

    # ---- shared scalar columns ----
    mx = (coords_xyz.max(0) + 1).astype(f64) * VS
    mn = (coords_xyz.min(0) - 1).astype(f64) * VS
    bias65 = np.zeros(65, f64)
    bias65[0:N_CLS] = sem_b.astype(f64)
    bias65[32:35] = c3
    bias65[64] = -cen_w.astype(f64).sum(0)[0]

    sc = np.zeros((C, 11), np.float32)
    sc[:, 0] = b1
    sc[:, 1] = bc
    sc[:, 2] = b2
    sc[0:65, 3] = bias65
    sc[32:35, 4] = mn
    sc[32:35, 5] = mx
    sc[:, 6] = b1 + 1.0
    sc[:, 7] = bc + 1.0
    sc[:, 8] = b2 + 1.0

    nwb = 406 if zero_mask else 660
    wb = np.zeros((C, nwb), BF16)
    wb[:, 0:128] = W1.astype(BF16)
    wb[:, 128:256] = Wc.astype(BF16)
    wb[:, 256:384] = W2.astype(BF16)
    if zero_mask:
        wb[:, 384:402] = sem_w.astype(f64).astype(BF16)
        wb[:, 402:405] = W3.astype(BF16)
        wb[:, 405:406] = cen_w.astype(f64).astype(BF16)
    else:
        # j-major layout: rows [cls 0:18 | cen 18 | regJ 19:127]
        sc64 = scales.astype(f64)
        semJF = np.zeros((C, FOH), f64)
        fohF = np.zeros((C, FOH), f64)
        thrF = np.zeros(FOH, np.float32)
        biasF = np.zeros(FOH, np.float32)
        semJF[:, 0:N_CLS] = sem_w.astype(f64)
        fohF[:, 0:N_CLS] = cls_w.astype(f64)
        thrF[0:N_CLS] = LOGIT_THR - sem_b.astype(f64)
        biasF[0:N_CLS] = cls_b.astype(f64) - cls_w.astype(f64).sum(0)
        fohF[:, N_CLS] = cen_w.astype(f64)[:, 0]
        thrF[N_CLS] = -1e30
        biasF[N_CLS] = -cen_w.astype(f64).sum(0)[0]
        for j in range(N_REG):
            for cc in range(N_CLS):
                r = 19 + j * N_CLS + cc
                semJF[:, r] = sem_w.astype(f64)[:, cc]
                fohF[:, r] = reg_w.astype(f64)[:, j] * sc64[cc]
                thrF[r] = LOGIT_THR - sem_b.astype(f64)[cc]
                biasF[r] = -reg_w.astype(f64)[:, j].sum() * sc64[cc]
        wb[:, 384:511] = semJF.astype(BF16)
        wb[:, 511:638] = fohF.astype(BF16)
        wb[:, 638:641] = W3.astype(BF16)
        wb[:, 641:659] = sem_w.astype(f64).astype(BF16)
        wb[:, 659:660] = cen_w.astype(f64).astype(BF16)
        sc[0:FOH, 9] = thrF
        sc[0:FOH, 10] = biasF

    # ---- transposed, channel-major activations ----
    fT = np.ascontiguousarray(feats.T)
    gTf = np.ascontiguousarray(G.astype(np.float32).T)
    cT = coords_xyz.T.astype(np.float32) * VS

    wts = {"wb": wb, "sc": sc}
    in_maps = []
    for c in range(N_CORES):
        s, e = c * PER_CORE, (c + 1) * PER_CORE
        m = dict(wts)
        m["xT"] = np.ascontiguousarray(fT[:, s:e]).astype(BF16)
        m["gT"] = np.ascontiguousarray(gTf[:, s:e]).astype(BF16)
        m["cvs"] = np.ascontiguousarray(cT[:, s:e]).astype(BF16)
        in_maps.append(m)
    return in_maps, zero_mask


_CACHED = {}

# full path: regJ row 19 + j*18 + c  ->  output col 43 + c*6 + j
_REG_COLS = np.empty(FOH - 19, np.int64)
for _j in range(N_REG):
    for _c in range(N_CLS):
        _REG_COLS[_j * N_CLS + _c] = 43 + _c * N_REG + _j


def _assemble(res, zero_mask, n):
    """Map device outputs to reference layout [n, 151] f32."""
    o = np.zeros((n, OUT_ROWS), np.float32)
    outH = res["outH"][:, :n].astype(np.float32)
    o[:, 0:18] = outH[0:18].T
    o[:, 18:21] = outH[32:35].T
    o[:, 21:24] = res["outW"][:, :n].astype(np.float32).T
    o[:, 24] = outH[64]
    if not zero_mask:
        outB = res["outB"][:, :n].astype(np.float32)
        o[:, 25:43] = outB[0:N_CLS].T
        o[:, _REG_COLS] = outB[19:FOH].T
    return o


def kernel(**inputs):
    inputs = {k: np.asarray(v) for k, v in inputs.items()}
    in_maps, zero_mask = _host_prep(**inputs)
    ckey = "fast" if zero_mask else "full"
    if ckey not in _CACHED:
        _CACHED[ckey] = _build_fast() if zero_mask else _build_full()
    nc = _CACHED[ckey]
    res = run_bass_kernel_spmd(nc, in_maps, core_ids=list(range(N_CORES)))
    out = np.empty((N_VOX, OUT_ROWS), np.float32)
    for c in range(N_CORES):
        out[c * PER_CORE:(c + 1) * PER_CORE] = _assemble(
            res.results[c], zero_mask, PER_CORE)
    return out


# revision 15
# speedup vs baseline: 2.1359x; 1.0009x over previous
"""CAGroup3DHead kernel for 8 Trainium2 NeuronCores.

Strategy (data-parallel over voxels, per the sharding hint):
  - Host: integer index work (sorted-key neighbor lookup identical to the
    reference), weight fusion (BN folded into weights, ELU+1 bias shifts),
    and sharding marshaling (transpose to channel-major, bf16 cast,
    per-core slices).  The 3x3x3 sparse conv collapses to a gather: the
    (0,0,0) tap always hits, so conv_in = feats[rep]; the rare other-tap
    hits are folded into conv_in via W_k @ W_13^{-1} so the device conv is
    one dense matmul.
  - Semantic gate fast path: cls and reg_pc are gated by
    sigmoid(sem) > 0.15, i.e. sem_logit + sem_b > logit(0.15).  The host
    checks the rigorous bound max_i ||feats_i|| * max_c ||sem_w_c|| +
    sem_b_c < logit(0.15); when it holds (it does for the detection-prior
    bias -4.595 used here), every mask entry is exactly zero, so cls and
    reg_pc are exactly zero and the device skips them.  If the bound ever
    fails, a full device path (mask + masked heads) is built instead.
  - Device fast path (identical SPMD program on 8 cores): inputs are
    SBUF-resident (a few large DMAs); work proceeds in 1024-column
    supertiles.  Three 128x128 bf16 matmul chains (mlp1, conv, mlp2) with
    ELU+1 done as one scalar Exp + one relu + one min (exact); the three
    skinny heads (sem 18, voff 3, cen 1) share one [65,T] PSUM tile via
    32-row tile_position strips and drain with a single scalar op.
    Outputs are bf16 and stream out per-supertile; the host re-transposes
    and fills the zero sections during unsharding.
"""

import numpy as np
import ml_dtypes

import concourse.bass as bass
import concourse.bacc as bacc
import concourse.tile as tile
from concourse import mybir
from concourse.bass_utils import run_bass_kernel_spmd

BF16 = ml_dtypes.bfloat16

N_VOX = 100000
C = 128
N_CLS = 18
N_REG = 6
VS = 0.04
THR = 0.15
HASH_D = 260
N_CORES = 8
PER_CORE = N_VOX // N_CORES          # 12500
ST = 1024                            # supertile columns
N_ST = 13                            # 12 full + tail of 212
LOGIT_THR = float(np.log(THR / (1.0 - THR)))   # -1.734601..

FOH = 127                            # full-path head rows [cls|cen|regJ]
OUT_ROWS = 151

F32 = mybir.dt.float32
BF = mybir.dt.bfloat16
AOp = mybir.AluOpType
Act = mybir.ActivationFunctionType


def _build_fast():
    """Fast path: masks provably all-zero; compute sem/voff/voted/cen only."""
    nc = bacc.Bacc(trn_type="TRN2")

    xT_d = nc.dram_tensor("xT", [C, PER_CORE], BF, kind="ExternalInput")
    gT_d = nc.dram_tensor("gT", [C, PER_CORE], BF, kind="ExternalInput")
    cvs_d = nc.dram_tensor("cvs", [3, PER_CORE], BF, kind="ExternalInput")
    # bf16 weights: w1 0:128, wc 128:256, w2 256:384, sem 384:402,
    # w3 402:405, cen 405:406
    wb_d = nc.dram_tensor("wb", [C, 406], BF, kind="ExternalInput")
    # f32 per-partition scalars [128, 11]: col0 b1, col1 bc, col2 b2,
    # col3 bias65 (rows 0:18 semb, 32:35 c3, 64 cenb),
    # col4 mn (rows 32:35), col5 mx (rows 32:35), col6-8 b1/bc/b2 + 1
    sc_d = nc.dram_tensor("sc", [C, 11], F32, kind="ExternalInput")
    outH_d = nc.dram_tensor("outH", [65, PER_CORE], BF, kind="ExternalOutput")
    outW_d = nc.dram_tensor("outW", [3, PER_CORE], BF, kind="ExternalOutput")

    chunks = [(0, 1024), (1024, 3072), (3072, 6144), (6144, 9216),
              (9216, PER_CORE)]

    with tile.TileContext(nc) as tc:
        with (
            tc.tile_pool(name="res", bufs=1) as res,
            tc.tile_pool(name="epool", bufs=4) as epool,
            tc.tile_pool(name="rpool", bufs=4) as rpool,
            tc.tile_pool(name="fpool", bufs=4) as fpool,
            tc.tile_pool(name="hpool", bufs=3) as hpool,
            tc.tile_pool(name="pbig", bufs=3, space=bass.MemorySpace.PSUM) as pbig,
            tc.tile_pool(name="phead", bufs=1, space=bass.MemorySpace.PSUM) as ph,
        ):
            wb = res.tile([C, 406], BF)
            sc = res.tile([C, 11], F32)
            nc.sync.dma_start(wb[:], wb_d[:])
            nc.sync.dma_start(sc[:], sc_d[:])
            w1 = wb[:, 0:128]
            wc = wb[:, 128:256]
            w2 = wb[:, 256:384]
            semw = wb[:, 384:402]
            w3 = wb[:, 402:405]
            cenw = wb[:, 405:406]
            b1 = sc[:, 0:1]
            bc = sc[:, 1:2]
            b2 = sc[:, 2:3]
            bias65 = sc[0:65, 3:4]
            mn3 = sc[32:35, 4:5]
            mx3 = sc[32:35, 5:6]
            b1p1 = sc[:, 6:7]
            bcp1 = sc[:, 7:8]
            b2p1 = sc[:, 8:9]

            xT = res.tile([C, PER_CORE], BF)
            gT = res.tile([C, PER_CORE], BF)
            cvsr = res.tile([35, PER_CORE], BF)
            for a, b in chunks:
                nc.sync.dma_start(xT[:, a:b], xT_d[:, a:b])
                nc.sync.dma_start(gT[:, a:b], gT_d[:, a:b])
                nc.sync.dma_start(cvsr[32:35, a:b], cvs_d[:, a:b])

            def mm_blocks(p, w, src_sb, off, L, rows=None, tp=None):
                for a in range(0, L, 512):
                    e = min(a + 512, L)
                    dst = p[:, a:e] if rows is None else p[rows[0]:rows[1], a:e]
                    if tp is None:
                        nc.tensor.matmul(dst, w, src_sb[:, off + a:off + e],
                                         start=True, stop=True)
                    else:
                        nc.tensor.matmul(dst, w, src_sb[:, off + a:off + e],
                                         start=True, stop=True,
                                         tile_position=tp)

            def elu_tail(p, bias, biasp1, L, scalar_relu=False):
                # f = min(max(y+b+1, 1), exp(y+b)) == ELU(y+b) + 1, exact
                et = epool.tile([C, ST], BF, tag="e")
                nc.scalar.activation(et[:, :L], p[:, :L], Act.Exp, bias=bias)
                ft = fpool.tile([C, ST], BF, tag="f")
                if scalar_relu:
                    # relu on Scalar, (r+1) min e on Vector (STT)
                    rt = rpool.tile([C, ST], BF, tag="r")
                    nc.scalar.activation(rt[:, :L], p[:, :L], Act.Relu,
                                         bias=bias)
                    nc.vector.scalar_tensor_tensor(
                        ft[:, :L], rt[:, :L], 1.0, et[:, :L],
                        AOp.add, AOp.min)
                else:
                    # max(y+b+1, 1) on Vector (TS), min on Vector (TT, 2x)
                    rt = rpool.tile([C, ST], BF, tag="r")
                    nc.vector.tensor_scalar(rt[:, :L], p[:, :L], biasp1, 1.0,
                                            AOp.add, AOp.max)
                    nc.vector.tensor_tensor(ft[:, :L], rt[:, :L], et[:, :L],
                                            AOp.min)
                return ft

            def head_tail(st):
                # finish supertile st: all three skinny head matmuls grouped
                # back-to-back (deps long ready; disjoint col strips), then
                # drain + vp; the voted clip is deferred further (gpsimd
                # latency must not head-of-line-block the vector queue)
                cs0, L0, f20, fo0 = st
                phd0 = ph.tile([65, ST], F32, tag="ph")
                for a in range(0, L0, 512):
                    e = min(a + 512, L0)
                    nc.tensor.matmul(phd0[0:18, a:e], semw,
                                     xT[:, cs0 + a:cs0 + e],
                                     start=True, stop=True,
                                     tile_position=(0, 0))
                    nc.tensor.matmul(phd0[32:35, a:e], w3, f20[:, a:e],
                                     start=True, stop=True,
                                     tile_position=(0, 32))
                    nc.tensor.matmul(phd0[64:65, a:e], cenw, fo0[:, a:e],
                                     start=True, stop=True,
                                     tile_position=(0, 64))
                sv = hpool.tile([65, ST], BF, tag="sv")
                nc.scalar.activation(sv[:, :L0], phd0[:, :L0], Act.Identity,
                                     bias=bias65)
                nc.sync.dma_start(outH_d[:, cs0:cs0 + L0], sv[:, :L0])
                vp = hpool.tile([35, ST], BF, tag="vp")
                nc.gpsimd.tensor_tensor(vp[32:35, :L0], sv[32:35, :L0],
                                        cvsr[32:35, cs0:cs0 + L0], AOp.add)
                return (cs0, L0, vp)

            def voted_tail(vst):
                cs0, L0, vp = vst
                vt = hpool.tile([35, ST], BF, tag="vt")
                nc.vector.tensor_scalar(vt[32:35, :L0], vp[32:35, :L0],
                                        mn3, mx3, AOp.max, AOp.min)
                nc.sync.dma_start(outW_d[:, cs0:cs0 + L0], vt[32:35, :L0])

            pending = None
            for k in range(N_ST):
                cs = k * ST
                L = min(ST, PER_CORE - cs)

                # mlp1 + conv matmuls, then sem (xT-only) early for overlap
                p1 = pbig.tile([C, ST], F32, tag="p")
                mm_blocks(p1, w1, xT, cs, L)
                pc = pbig.tile([C, ST], F32, tag="p")
                mm_blocks(pc, wc, gT, cs, L)
                vpending = head_tail(pending) if pending is not None else None

                f1 = elu_tail(p1, b1, b1p1, L)
                fo = elu_tail(pc, bc, bcp1, L, scalar_relu=True)

                p2 = pbig.tile([C, ST], F32, tag="p")
                mm_blocks(p2, w2, f1, 0, L)
                f2 = elu_tail(p2, b2, b2p1, L)
                if vpending is not None:
                    voted_tail(vpending)
                pending = (cs, L, f2, fo)

            voted_tail(head_tail(pending))

    nc.finalize()
    return nc


def _build_full():
    """Fallback: full mask + masked-heads path (used if the zero-mask
    bound fails).  j-major reg layout, one is_gt + one STT for heads."""
    nc = bacc.Bacc(trn_type="TRN2")

    xT_d = nc.dram_tensor("xT", [C, PER_CORE], BF, kind="ExternalInput")
    gT_d = nc.dram_tensor("gT", [C, PER_CORE], BF, kind="ExternalInput")
    cvs_d = nc.dram_tensor("cvs", [3, PER_CORE], BF, kind="ExternalInput")
    # w1 0:128, wc 128:256, w2 256:384, semJF 384:511, fohF 511:638,
    # w3 638:641, sem 641:659, cen 659:660
    wb_d = nc.dram_tensor("wb", [C, 660], BF, kind="ExternalInput")
    # col0 b2, col1 bias65, col2 mn, col3 mx, col4 thrF, col5 biasF
    sc_d = nc.dram_tensor("sc", [C, 8], F32, kind="ExternalInput")
    outH_d = nc.dram_tensor("outH", [65, PER_CORE], BF, kind="ExternalOutput")
    outW_d = nc.dram_tensor("outW", [3, PER_CORE], BF, kind="ExternalOutput")
    outB_d = nc.dram_tensor("outB", [FOH, PER_CORE], BF, kind="ExternalOutput")

    chunks = [(0, 3072), (3072, 6144), (6144, 9216), (9216, PER_CORE)]

    with tile.TileContext(nc) as tc:
        with (
            tc.tile_pool(name="res", bufs=1) as res,
            tc.tile_pool(name="epool", bufs=4) as epool,
            tc.tile_pool(name="rpool", bufs=4) as rpool,
            tc.tile_pool(name="fpool", bufs=4) as fpool,
            tc.tile_pool(name="hpool", bufs=3) as hpool,
            tc.tile_pool(name="mpool", bufs=2) as mpool,
            tc.tile_pool(name="pbig", bufs=2, space=bass.MemorySpace.PSUM) as pbig,
            tc.tile_pool(name="phead", bufs=1, space=bass.MemorySpace.PSUM) as ph,
            tc.tile_pool(name="pfoh", bufs=1, space=bass.MemorySpace.PSUM) as pf,
        ):
            wb = res.tile([C, 660], BF)
            sc = res.tile([C, 11], F32)
            nc.sync.dma_start(wb[:], wb_d[:])
            nc.sync.dma_start(sc[:], sc_d[:])
            w1 = wb[:, 0:128]
            wc = wb[:, 128:256]
            w2 = wb[:, 256:384]
            semJF = wb[:, 384:511]
            fohF = wb[:, 511:638]
            w3 = wb[:, 638:641]
            semw = wb[:, 641:659]
            cenw = wb[:, 659:660]
            b1 = sc[:, 0:1]
            bc = sc[:, 1:2]
            b2 = sc[:, 2:3]
            bias65 = sc[0:65, 3:4]
            mn3 = sc[32:35, 4:5]
            mx3 = sc[32:35, 5:6]
            b1p1 = sc[:, 6:7]
            bcp1 = sc[:, 7:8]
            b2p1 = sc[:, 8:9]
            thrF = sc[0:FOH, 9:10]
            biasF = sc[0:FOH, 10:11]

            xT = res.tile([C, PER_CORE], BF)
            gT = res.tile([C, PER_CORE], BF)
            cvsr = res.tile([35, PER_CORE], BF)
            for a, b in chunks:
                nc.sync.dma_start(xT[:, a:b], xT_d[:, a:b])
                nc.sync.dma_start(gT[:, a:b], gT_d[:, a:b])
                nc.sync.dma_start(cvsr[32:35, a:b], cvs_d[:, a:b])

            for k in range(N_ST):
                cs = k * ST
                L = min(ST, PER_CORE - cs)

                def elu_layer(w, src_sb, off, bias):
                    p = pbig.tile([C, ST], F32, tag="p")
                    for a in range(0, L, 512):
                        e = min(a + 512, L)
                        nc.tensor.matmul(p[:, a:e], w,
                                         src_sb[:, off + a:off + e],
                                         start=True, stop=True)
                    et = epool.tile([C, ST], BF, tag="e")
                    nc.scalar.activation(et[:, :L], p[:, :L], Act.Exp,
                                         bias=bias)
                    rt = rpool.tile([C, ST], BF, tag="r")
                    nc.vector.tensor_scalar(rt[:, :L], p[:, :L], bias,
                                            0.0, AOp.add, AOp.max)
                    ft = fpool.tile([C, ST], BF, tag="f")
                    nc.vector.scalar_tensor_tensor(
                        ft[:, :L], rt[:, :L], 1.0, et[:, :L],
                        AOp.add, AOp.min)
                    return ft

                f1 = elu_layer(w1, xT, cs, b1)
                fo = elu_layer(wc, gT, cs, bc)
                f2 = elu_layer(w2, f1, 0, b2)

                phd = ph.tile([65, ST], F32, tag="ph")
                for a in range(0, L, 512):
                    e = min(a + 512, L)
                    nc.tensor.matmul(phd[0:18, a:e], semw,
                                     xT[:, cs + a:cs + e],
                                     start=True, stop=True,
                                     tile_position=(0, 0))
                    nc.tensor.matmul(phd[32:35, a:e], w3, f2[:, a:e],
                                     start=True, stop=True,
                                     tile_position=(0, 32))
                    nc.tensor.matmul(phd[64:65, a:e], cenw, fo[:, a:e],
                                     start=True, stop=True,
                                     tile_position=(0, 64))
                sv = hpool.tile([65, ST], BF, tag="sv")
                nc.scalar.activation(sv[:, :L], phd[:, :L], Act.Identity,
                                     bias=bias65)
                nc.sync.dma_start(outH_d[:, cs:cs + L], sv[:, :L])

                vp = hpool.tile([35, ST], BF, tag="vp")
                nc.gpsimd.tensor_tensor(vp[32:35, :L], sv[32:35, :L],
                                        cvsr[32:35, cs:cs + L], AOp.add)
                vt = hpool.tile([35, ST], BF, tag="vt")
                nc.vector.tensor_scalar(vt[32:35, :L], vp[32:35, :L],
                                        mn3, mx3, AOp.max, AOp.min)
                nc.sync.dma_start(outW_d[:, cs:cs + L], vt[32:35, :L])

                # masked heads: (foh + biasF) * (semJ > thrF)
                psem = ph.tile([FOH, ST], F32, tag="ph")
                for a in range(0, L, 512):
                    e = min(a + 512, L)
                    nc.tensor.matmul(psem[:, a:e], semJF,
                                     xT[:, cs + a:cs + e],
                                     start=True, stop=True)
                maskF = mpool.tile([FOH, ST], BF, tag="maskF")
                nc.vector.tensor_scalar(maskF[:, :L], psem[:, :L], thrF,
                                        None, AOp.is_gt)
                pfo = pf.tile([FOH, ST], F32, tag="pfoh")
                for a in range(0, L, 512):
                    e = min(a + 512, L)
                    nc.tensor.matmul(pfo[:, a:e], fohF, fo[:, a:e],
                                     start=True, stop=True)
                hb = mpool.tile([FOH, ST], BF, tag="hb")
                nc.vector.scalar_tensor_tensor(
                    hb[:, :L], pfo[:, :L], biasF, maskF[:, :L],
                    AOp.add, AOp.mult)
                nc.sync.dma_start(outB_d[:, cs:cs + L], hb[:, :L])

    nc.finalize()
    return nc


def _host_prep(feats, coords_xyz, batch_idx,
               off_w1, off_g1, off_b1, off_w2, off_g2, off_b2, off_w3,
               fo_w, fo_g, fo_b, sem_w, sem_b, cen_w, cls_w, cls_b, reg_w,
               scales):
    f64 = np.float64
    N = feats.shape[0]

    # ---- zero-mask bound: |x.w_c| <= max||x|| * ||w_c|| (Cauchy-Schwarz)
    feats64 = feats.astype(f64)
    max_row = np.sqrt((feats64 * feats64).sum(1).max())
    colnrm = np.sqrt((sem_w.astype(f64) ** 2).sum(0))
    zero_mask = bool(
        np.all(max_row * colnrm + sem_b.astype(f64) < LOGIT_THR - 1e-3))

    # ---- neighbor lookup (identical to reference's sorted-key search) ----
    c1 = coords_xyz.astype(np.int64) + 1
    key = ((batch_idx.astype(np.int64) * HASH_D + c1[:, 0]) * HASH_D
           + c1[:, 1]) * HASH_D + c1[:, 2]
    order = np.argsort(key, kind="stable")
    skey = key[order]
    pos = np.searchsorted(skey, key)
    rep = order[pos]                      # first voxel with same key

    # ---- fused weights (BN folded; ELU+1 handled via bias shifts) ----
    W1 = off_w1.astype(f64) * off_g1.astype(f64)[None, :]
    b1 = off_b1.astype(f64)
    W2 = off_w2.astype(f64) * off_g2.astype(f64)[None, :]
    b2 = off_b2.astype(f64) - W2.sum(0)
    W3 = off_w3.astype(f64)
    c3 = -W3.sum(0)
    Wc = fo_w[13].astype(f64) * fo_g.astype(f64)[None, :]
    bc = fo_b.astype(f64)

    # ---- conv input: gather + fold rare non-center taps via Wc13^-1 ----
    G = feats64[rep]
    Winv = np.linalg.inv(fo_w[13].astype(f64))
    k = 0
    for dx in (-1, 0, 1):
        for dy in (-1, 0, 1):
            for dz in (-1, 0, 1):
                if (dx, dy, dz) != (0, 0, 0):
                    nk = key + (dx * HASH_D + dy) * HASH_D + dz
                    p = np.clip(np.searchsorted(skey, nk), 0, N - 1)
                    hit = skey[p] == nk
                    if hit.any():
                        dst = np.nonzero(hit)[0]
                        src = order[p[hit]]
                        A = fo_w[k].astype(f64) @ Winv
                        np.add.at(G, dst, feats64[src] @ A)
                k += 1

    # ---- shared scalar columns ----
    mx = (coords_xyz.max(0) + 1).astype(f64) * VS
    mn = (coords_xyz.min(0) - 1).astype(f64) * VS
    bias65 = np.zeros(65, f64)
    bias65[0:N_CLS] = sem_b.astype(f64)
    bias65[32:35] = c3
    bias65[64] = -cen_w.astype(f64).sum(0)[0]

    sc = np.zeros((C, 11), np.float32)
    sc[:, 0] = b1
    sc[:, 1] = bc
    sc[:, 2] = b2
    sc[0:65, 3] = bias65
    sc[32:35, 4] = mn
    sc[32:35, 5] = mx
    sc[:, 6] = b1 + 1.0
    sc[:, 7] = bc + 1.0
    sc[:, 8] = b2 + 1.0

    nwb = 406 if zero_mask else 660
    wb = np.zeros((C, nwb), BF16)
    wb[:, 0:128] = W1.astype(BF16)
    wb[:, 128:256] = Wc.astype(BF16)
    wb[:, 256:384] = W2.astype(BF16)
    if zero_mask:
        wb[:, 384:402] = sem_w.astype(f64).astype(BF16)
        wb[:, 402:405] = W3.astype(BF16)
        wb[:, 405:406] = cen_w.astype(f64).astype(BF16)
    else:
        # j-major layout: rows [cls 0:18 | cen 18 | regJ 19:127]
        sc64 = scales.astype(f64)
        semJF = np.zeros((C, FOH), f64)
        fohF = np.zeros((C, FOH), f64)
        thrF = np.zeros(FOH, np.float32)
        biasF = np.zeros(FOH, np.float32)
        semJF[:, 0:N_CLS] = sem_w.astype(f64)
        fohF[:, 0:N_CLS] = cls_w.astype(f64)
        thrF[0:N_CLS] = LOGIT_THR - sem_b.astype(f64)
        biasF[0:N_CLS] = cls_b.astype(f64) - cls_w.astype(f64).sum(0)
        fohF[:, N_CLS] = cen_w.astype(f64)[:, 0]
        thrF[N_CLS] = -1e30
        biasF[N_CLS] = -cen_w.astype(f64).sum(0)[0]
        for j in range(N_REG):
            for cc in range(N_CLS):
                r = 19 + j * N_CLS + cc
                semJF[:, r] = sem_w.astype(f64)[:, cc]
                fohF[:, r] = reg_w.astype(f64)[:, j] * sc64[cc]
                thrF[r] = LOGIT_THR - sem_b.astype(f64)[cc]
                biasF[r] = -reg_w.astype(f64)[:, j].sum() * sc64[cc]
        wb[:, 384:511] = semJF.astype(BF16)
        wb[:, 511:638] = fohF.astype(BF16)
        wb[:, 638:641] = W3.astype(BF16)
        wb[:, 641:659] = sem_w.astype(f64).astype(BF16)
        wb[:, 659:660] = cen_w.astype(f64).astype(BF16)
        sc[0:FOH, 9] = thrF
        sc[0:FOH, 10] = biasF

    # ---- transposed, channel-major activations ----
    fT = np.ascontiguousarray(feats.T)
    gTf = np.ascontiguousarray(G.astype(np.float32).T)
    cT = coords_xyz.T.astype(np.float32) * VS

    wts = {"wb": wb, "sc": sc}
    in_maps = []
    for c in range(N_CORES):
        s, e = c * PER_CORE, (c + 1) * PER_CORE
        m = dict(wts)
        m["xT"] = np.ascontiguousarray(fT[:, s:e]).astype(BF16)
        m["gT"] = np.ascontiguousarray(gTf[:, s:e]).astype(BF16)
        m["cvs"] = np.ascontiguousarray(cT[:, s:e]).astype(BF16)
        in_maps.append(m)
    return in_maps, zero_mask


_CACHED = {}

# full path: regJ row 19 + j*18 + c  ->  output col 43 + c*6 + j
_REG_COLS = np.empty(FOH - 19, np.int64)
for _j in range(N_REG):
    for _c in range(N_CLS):
        _REG_COLS[_j * N_CLS + _c] = 43 + _c * N_REG + _j


def _assemble(res, zero_mask, n):
    """Map device outputs to reference layout [n, 151] f32."""
    o = np.zeros((n, OUT_ROWS), np.float32)
    outH = res["outH"][:, :n].astype(np.float32)
    o[:, 0:18] = outH[0:18].T
    o[:, 18:21] = outH[32:35].T
    o[:, 21:24] = res["outW"][:, :n].astype(np.float32).T
    o[:, 24] = outH[64]
    if not zero_mask:
        outB = res["outB"][:, :n].astype(np.float32)
        o[:, 25:43] = outB[0:N_CLS].T
        o[:, _REG_COLS] = outB[19:FOH].T
    return o


def kernel(**inputs):
    inputs = {k: np.asarray(v) for k, v in inputs.items()}
    in_maps, zero_mask = _host_prep(**inputs)
    ckey = "fast" if zero_mask else "full"
    if ckey not in _CACHED:
        _CACHED[ckey] = _build_fast() if zero_mask else _build_full()
    nc = _CACHED[ckey]
    res = run_bass_kernel_spmd(nc, in_maps, core_ids=list(range(N_CORES)))
    out = np.empty((N_VOX, OUT_ROWS), np.float32)
    for c in range(N_CORES):
        out[c * PER_CORE:(c + 1) * PER_CORE] = _assemble(
            res.results[c], zero_mask, PER_CORE)
    return out


# revision 16
# speedup vs baseline: 2.1451x; 1.0043x over previous
"""CAGroup3DHead kernel for 8 Trainium2 NeuronCores.

Strategy (data-parallel over voxels, per the sharding hint):
  - Host: integer index work (sorted-key neighbor lookup identical to the
    reference), weight fusion (BN folded into weights, ELU+1 bias shifts),
    and sharding marshaling (transpose to channel-major, bf16 cast,
    per-core slices).  The 3x3x3 sparse conv collapses to a gather: the
    (0,0,0) tap always hits, so conv_in = feats[rep]; the rare other-tap
    hits are folded into conv_in via W_k @ W_13^{-1} so the device conv is
    one dense matmul.
  - Semantic gate fast path: cls and reg_pc are gated by
    sigmoid(sem) > 0.15, i.e. sem_logit + sem_b > logit(0.15).  The host
    checks the rigorous bound max_i ||feats_i|| * max_c ||sem_w_c|| +
    sem_b_c < logit(0.15); when it holds (it does for the detection-prior
    bias -4.595 used here), every mask entry is exactly zero, so cls and
    reg_pc are exactly zero and the device skips them.  If the bound ever
    fails, a full device path (mask + masked heads) is built instead.
  - Device fast path (identical SPMD program on 8 cores): inputs are
    SBUF-resident (a few large DMAs); work proceeds in 1024-column
    supertiles.  Three 128x128 bf16 matmul chains (mlp1, conv, mlp2) with
    ELU+1 done as one scalar Exp + one relu + one min (exact); the three
    skinny heads (sem 18, voff 3, cen 1) share one [65,T] PSUM tile via
    32-row tile_position strips and drain with a single scalar op.
    Outputs are bf16 and stream out per-supertile; the host re-transposes
    and fills the zero sections during unsharding.
"""

import numpy as np
import ml_dtypes

import concourse.bass as bass
import concourse.bacc as bacc
import concourse.tile as tile
from concourse import mybir
from concourse.bass_utils import run_bass_kernel_spmd

BF16 = ml_dtypes.bfloat16

N_VOX = 100000
C = 128
N_CLS = 18
N_REG = 6
VS = 0.04
THR = 0.15
HASH_D = 260
N_CORES = 8
PER_CORE = N_VOX // N_CORES          # 12500
ST = 1024                            # supertile columns
N_ST = 13                            # 12 full + tail of 212
LOGIT_THR = float(np.log(THR / (1.0 - THR)))   # -1.734601..

FOH = 127                            # full-path head rows [cls|cen|regJ]
OUT_ROWS = 151

F32 = mybir.dt.float32
BF = mybir.dt.bfloat16
AOp = mybir.AluOpType
Act = mybir.ActivationFunctionType


def _build_fast():
    """Fast path: masks provably all-zero; compute sem/voff/voted/cen only."""
    nc = bacc.Bacc(trn_type="TRN2")

    xT_d = nc.dram_tensor("xT", [C, PER_CORE], BF, kind="ExternalInput")
    gT_d = nc.dram_tensor("gT", [C, PER_CORE], BF, kind="ExternalInput")
    cvs_d = nc.dram_tensor("cvs", [3, PER_CORE], BF, kind="ExternalInput")
    # bf16 weights: w1 0:128, wc 128:256, w2 256:384, sem 384:402,
    # w3 402:405, cen 405:406
    wb_d = nc.dram_tensor("wb", [C, 406], BF, kind="ExternalInput")
    # f32 per-partition scalars [128, 11]: col0 b1, col1 bc, col2 b2,
    # col3 bias65 (rows 0:18 semb, 32:35 c3, 64 cenb),
    # col4 mn (rows 32:35), col5 mx (rows 32:35), col6-8 b1/bc/b2 + 1
    sc_d = nc.dram_tensor("sc", [C, 11], F32, kind="ExternalInput")
    outH_d = nc.dram_tensor("outH", [65, PER_CORE], BF, kind="ExternalOutput")
    outW_d = nc.dram_tensor("outW", [3, PER_CORE], BF, kind="ExternalOutput")

    chunks = [(0, 512), (512, 1024), (1024, 3072), (3072, 6144),
              (6144, 9216), (9216, PER_CORE)]

    with tile.TileContext(nc) as tc:
        with (
            tc.tile_pool(name="res", bufs=1) as res,
            tc.tile_pool(name="epool", bufs=4) as epool,
            tc.tile_pool(name="rpool", bufs=4) as rpool,
            tc.tile_pool(name="fpool", bufs=4) as fpool,
            tc.tile_pool(name="hpool", bufs=3) as hpool,
            tc.tile_pool(name="pbig", bufs=3, space=bass.MemorySpace.PSUM) as pbig,
            tc.tile_pool(name="phead", bufs=1, space=bass.MemorySpace.PSUM) as ph,
        ):
            wb = res.tile([C, 406], BF)
            sc = res.tile([C, 11], F32)
            nc.sync.dma_start(wb[:], wb_d[:])
            nc.sync.dma_start(sc[:], sc_d[:])
            w1 = wb[:, 0:128]
            wc = wb[:, 128:256]
            w2 = wb[:, 256:384]
            semw = wb[:, 384:402]
            w3 = wb[:, 402:405]
            cenw = wb[:, 405:406]
            b1 = sc[:, 0:1]
            bc = sc[:, 1:2]
            b2 = sc[:, 2:3]
            bias65 = sc[0:65, 3:4]
            mn3 = sc[32:35, 4:5]
            mx3 = sc[32:35, 5:6]
            b1p1 = sc[:, 6:7]
            bcp1 = sc[:, 7:8]
            b2p1 = sc[:, 8:9]

            xT = res.tile([C, PER_CORE], BF)
            gT = res.tile([C, PER_CORE], BF)
            cvsr = res.tile([35, PER_CORE], BF)
            for a, b in chunks:
                nc.sync.dma_start(xT[:, a:b], xT_d[:, a:b])
                nc.sync.dma_start(gT[:, a:b], gT_d[:, a:b])
                if a >= 1024:
                    nc.sync.dma_start(cvsr[32:35, a:b], cvs_d[:, a:b])
            # cvs cols 0:1024 last: not needed until supertile 0's head_tail
            nc.sync.dma_start(cvsr[32:35, 0:1024], cvs_d[:, 0:1024])

            def mm_blocks(p, w, src_sb, off, L, rows=None, tp=None):
                for a in range(0, L, 512):
                    e = min(a + 512, L)
                    dst = p[:, a:e] if rows is None else p[rows[0]:rows[1], a:e]
                    if tp is None:
                        nc.tensor.matmul(dst, w, src_sb[:, off + a:off + e],
                                         start=True, stop=True)
                    else:
                        nc.tensor.matmul(dst, w, src_sb[:, off + a:off + e],
                                         start=True, stop=True,
                                         tile_position=tp)

            def elu_tail(p, bias, biasp1, L, scalar_relu=False):
                # f = min(max(y+b+1, 1), exp(y+b)) == ELU(y+b) + 1, exact
                et = epool.tile([C, ST], BF, tag="e")
                nc.scalar.activation(et[:, :L], p[:, :L], Act.Exp, bias=bias)
                ft = fpool.tile([C, ST], BF, tag="f")
                if scalar_relu:
                    # relu on Scalar, (r+1) min e on Vector (STT)
                    rt = rpool.tile([C, ST], BF, tag="r")
                    nc.scalar.activation(rt[:, :L], p[:, :L], Act.Relu,
                                         bias=bias)
                    nc.vector.scalar_tensor_tensor(
                        ft[:, :L], rt[:, :L], 1.0, et[:, :L],
                        AOp.add, AOp.min)
                else:
                    # max(y+b+1, 1) on Vector (TS), min on Vector (TT, 2x)
                    rt = rpool.tile([C, ST], BF, tag="r")
                    nc.vector.tensor_scalar(rt[:, :L], p[:, :L], biasp1, 1.0,
                                            AOp.add, AOp.max)
                    nc.vector.tensor_tensor(ft[:, :L], rt[:, :L], et[:, :L],
                                            AOp.min)
                return ft

            def head_tail(st):
                # finish supertile st: all three skinny head matmuls grouped
                # back-to-back (deps long ready; disjoint col strips), then
                # drain + vp; the voted clip is deferred further (gpsimd
                # latency must not head-of-line-block the vector queue)
                cs0, L0, f20, fo0 = st
                phd0 = ph.tile([65, ST], F32, tag="ph")
                for a in range(0, L0, 512):
                    e = min(a + 512, L0)
                    nc.tensor.matmul(phd0[0:18, a:e], semw,
                                     xT[:, cs0 + a:cs0 + e],
                                     start=True, stop=True,
                                     tile_position=(0, 0))
                    nc.tensor.matmul(phd0[32:35, a:e], w3, f20[:, a:e],
                                     start=True, stop=True,
                                     tile_position=(0, 32))
                    nc.tensor.matmul(phd0[64:65, a:e], cenw, fo0[:, a:e],
                                     start=True, stop=True,
                                     tile_position=(0, 64))
                sv = hpool.tile([65, ST], BF, tag="sv")
                nc.scalar.activation(sv[:, :L0], phd0[:, :L0], Act.Identity,
                                     bias=bias65)
                nc.sync.dma_start(outH_d[:, cs0:cs0 + L0], sv[:, :L0])
                vp = hpool.tile([35, ST], BF, tag="vp")
                nc.gpsimd.tensor_tensor(vp[32:35, :L0], sv[32:35, :L0],
                                        cvsr[32:35, cs0:cs0 + L0], AOp.add)
                return (cs0, L0, vp)

            def voted_tail(vst):
                cs0, L0, vp = vst
                vt = hpool.tile([35, ST], BF, tag="vt")
                nc.vector.tensor_scalar(vt[32:35, :L0], vp[32:35, :L0],
                                        mn3, mx3, AOp.max, AOp.min)
                nc.sync.dma_start(outW_d[:, cs0:cs0 + L0], vt[32:35, :L0])

            pending = None
            for k in range(N_ST):
                cs = k * ST
                L = min(ST, PER_CORE - cs)

                # mlp1 + conv matmuls, then sem (xT-only) early for overlap
                p1 = pbig.tile([C, ST], F32, tag="p")
                mm_blocks(p1, w1, xT, cs, L)
                pc = pbig.tile([C, ST], F32, tag="p")
                mm_blocks(pc, wc, gT, cs, L)
                vpending = head_tail(pending) if pending is not None else None

                f1 = elu_tail(p1, b1, b1p1, L)
                fo = elu_tail(pc, bc, bcp1, L, scalar_relu=True)

                p2 = pbig.tile([C, ST], F32, tag="p")
                mm_blocks(p2, w2, f1, 0, L)
                f2 = elu_tail(p2, b2, b2p1, L)
                if vpending is not None:
                    voted_tail(vpending)
                pending = (cs, L, f2, fo)

            voted_tail(head_tail(pending))

    nc.finalize()
    return nc


def _build_full():
    """Fallback: full mask + masked-heads path (used if the zero-mask
    bound fails).  j-major reg layout, one is_gt + one STT for heads."""
    nc = bacc.Bacc(trn_type="TRN2")

    xT_d = nc.dram_tensor("xT", [C, PER_CORE], BF, kind="ExternalInput")
    gT_d = nc.dram_tensor("gT", [C, PER_CORE], BF, kind="ExternalInput")
    cvs_d = nc.dram_tensor("cvs", [3, PER_CORE], BF, kind="ExternalInput")
    # w1 0:128, wc 128:256, w2 256:384, semJF 384:511, fohF 511:638,
    # w3 638:641, sem 641:659, cen 659:660
    wb_d = nc.dram_tensor("wb", [C, 660], BF, kind="ExternalInput")
    # col0 b2, col1 bias65, col2 mn, col3 mx, col4 thrF, col5 biasF
    sc_d = nc.dram_tensor("sc", [C, 8], F32, kind="ExternalInput")
    outH_d = nc.dram_tensor("outH", [65, PER_CORE], BF, kind="ExternalOutput")
    outW_d = nc.dram_tensor("outW", [3, PER_CORE], BF, kind="ExternalOutput")
    outB_d = nc.dram_tensor("outB", [FOH, PER_CORE], BF, kind="ExternalOutput")

    chunks = [(0, 3072), (3072, 6144), (6144, 9216), (9216, PER_CORE)]

    with tile.TileContext(nc) as tc:
        with (
            tc.tile_pool(name="res", bufs=1) as res,
            tc.tile_pool(name="epool", bufs=4) as epool,
            tc.tile_pool(name="rpool", bufs=4) as rpool,
            tc.tile_pool(name="fpool", bufs=4) as fpool,
            tc.tile_pool(name="hpool", bufs=3) as hpool,
            tc.tile_pool(name="mpool", bufs=2) as mpool,
            tc.tile_pool(name="pbig", bufs=2, space=bass.MemorySpace.PSUM) as pbig,
            tc.tile_pool(name="phead", bufs=1, space=bass.MemorySpace.PSUM) as ph,
            tc.tile_pool(name="pfoh", bufs=1, space=bass.MemorySpace.PSUM) as pf,
        ):
            wb = res.tile([C, 660], BF)
            sc = res.tile([C, 11], F32)
            nc.sync.dma_start(wb[:], wb_d[:])
            nc.sync.dma_start(sc[:], sc_d[:])
            w1 = wb[:, 0:128]
            wc = wb[:, 128:256]
            w2 = wb[:, 256:384]
            semJF = wb[:, 384:511]
            fohF = wb[:, 511:638]
            w3 = wb[:, 638:641]
            semw = wb[:, 641:659]
            cenw = wb[:, 659:660]
            b1 = sc[:, 0:1]
            bc = sc[:, 1:2]
            b2 = sc[:, 2:3]
            bias65 = sc[0:65, 3:4]
            mn3 = sc[32:35, 4:5]
            mx3 = sc[32:35, 5:6]
            b1p1 = sc[:, 6:7]
            bcp1 = sc[:, 7:8]
            b2p1 = sc[:, 8:9]
            thrF = sc[0:FOH, 9:10]
            biasF = sc[0:FOH, 10:11]

            xT = res.tile([C, PER_CORE], BF)
            gT = res.tile([C, PER_CORE], BF)
            cvsr = res.tile([35, PER_CORE], BF)
            for a, b in chunks:
                nc.sync.dma_start(xT[:, a:b], xT_d[:, a:b])
                nc.sync.dma_start(gT[:, a:b], gT_d[:, a:b])
                nc.sync.dma_start(cvsr[32:35, a:b], cvs_d[:, a:b])

            for k in range(N_ST):
                cs = k * ST
                L = min(ST, PER_CORE - cs)

                def elu_layer(w, src_sb, off, bias):
                    p = pbig.tile([C, ST], F32, tag="p")
                    for a in range(0, L, 512):
                        e = min(a + 512, L)
                        nc.tensor.matmul(p[:, a:e], w,
                                         src_sb[:, off + a:off + e],
                                         start=True, stop=True)
                    et = epool.tile([C, ST], BF, tag="e")
                    nc.scalar.activation(et[:, :L], p[:, :L], Act.Exp,
                                         bias=bias)
                    rt = rpool.tile([C, ST], BF, tag="r")
                    nc.vector.tensor_scalar(rt[:, :L], p[:, :L], bias,
                                            0.0, AOp.add, AOp.max)
                    ft = fpool.tile([C, ST], BF, tag="f")
                    nc.vector.scalar_tensor_tensor(
                        ft[:, :L], rt[:, :L], 1.0, et[:, :L],
                        AOp.add, AOp.min)
                    return ft

                f1 = elu_layer(w1, xT, cs, b1)
                fo = elu_layer(wc, gT, cs, bc)
                f2 = elu_layer(w2, f1, 0, b2)

                phd = ph.tile([65, ST], F32, tag="ph")
                for a in range(0, L, 512):
                    e = min(a + 512, L)
                    nc.tensor.matmul(phd[0:18, a:e], semw,
                                     xT[:, cs + a:cs + e],
                                     start=True, stop=True,
                                     tile_position=(0, 0))
                    nc.tensor.matmul(phd[32:35, a:e], w3, f2[:, a:e],
                                     start=True, stop=True,
                                     tile_position=(0, 32))
                    nc.tensor.matmul(phd[64:65, a:e], cenw, fo[:, a:e],
                                     start=True, stop=True,
                                     tile_position=(0, 64))
                sv = hpool.tile([65, ST], BF, tag="sv")
                nc.scalar.activation(sv[:, :L], phd[:, :L], Act.Identity,
                                     bias=bias65)
                nc.sync.dma_start(outH_d[:, cs:cs + L], sv[:, :L])

                vp = hpool.tile([35, ST], BF, tag="vp")
                nc.gpsimd.tensor_tensor(vp[32:35, :L], sv[32:35, :L],
                                        cvsr[32:35, cs:cs + L], AOp.add)
                vt = hpool.tile([35, ST], BF, tag="vt")
                nc.vector.tensor_scalar(vt[32:35, :L], vp[32:35, :L],
                                        mn3, mx3, AOp.max, AOp.min)
                nc.sync.dma_start(outW_d[:, cs:cs + L], vt[32:35, :L])

                # masked heads: (foh + biasF) * (semJ > thrF)
                psem = ph.tile([FOH, ST], F32, tag="ph")
                for a in range(0, L, 512):
                    e = min(a + 512, L)
                    nc.tensor.matmul(psem[:, a:e], semJF,
                                     xT[:, cs + a:cs + e],
                                     start=True, stop=True)
                maskF = mpool.tile([FOH, ST], BF, tag="maskF")
                nc.vector.tensor_scalar(maskF[:, :L], psem[:, :L], thrF,
                                        None, AOp.is_gt)
                pfo = pf.tile([FOH, ST], F32, tag="pfoh")
                for a in range(0, L, 512):
                    e = min(a + 512, L)
                    nc.tensor.matmul(pfo[:, a:e], fohF, fo[:, a:e],
                                     start=True, stop=True)
                hb = mpool.tile([FOH, ST], BF, tag="hb")
                nc.vector.scalar_tensor_tensor(
                    hb[:, :L], pfo[:, :L], biasF, maskF[:, :L],
                    AOp.add, AOp.mult)
                nc.sync.dma_start(outB_d[:, cs:cs + L], hb[:, :L])

    nc.finalize()
    return nc


def _host_prep(feats, coords_xyz, batch_idx,
               off_w1, off_g1, off_b1, off_w2, off_g2, off_b2, off_w3,
               fo_w, fo_g, fo_b, sem_w, sem_b, cen_w, cls_w, cls_b, reg_w,
               scales):
    f64 = np.float64
    N = feats.shape[0]

    # ---- zero-mask bound: |x.w_c| <= max||x|| * ||w_c|| (Cauchy-Schwarz)
    feats64 = feats.astype(f64)
    max_row = np.sqrt((feats64 * feats64).sum(1).max())
    colnrm = np.sqrt((sem_w.astype(f64) ** 2).sum(0))
    zero_mask = bool(
        np.all(max_row * colnrm + sem_b.astype(f64) < LOGIT_THR - 1e-3))

    # ---- neighbor lookup (identical to reference's sorted-key search) ----
    c1 = coords_xyz.astype(np.int64) + 1
    key = ((batch_idx.astype(np.int64) * HASH_D + c1[:, 0]) * HASH_D
           + c1[:, 1]) * HASH_D + c1[:, 2]
    order = np.argsort(key, kind="stable")
    skey = key[order]
    pos = np.searchsorted(skey, key)
    rep = order[pos]                      # first voxel with same key

    # ---- fused weights (BN folded; ELU+1 handled via bias shifts) ----
    W1 = off_w1.astype(f64) * off_g1.astype(f64)[None, :]
    b1 = off_b1.astype(f64)
    W2 = off_w2.astype(f64) * off_g2.astype(f64)[None, :]
    b2 = off_b2.astype(f64) - W2.sum(0)
    W3 = off_w3.astype(f64)
    c3 = -W3.sum(0)
    Wc = fo_w[13].astype(f64) * fo_g.astype(f64)[None, :]
    bc = fo_b.astype(f64)

    # ---- conv input: gather + fold rare non-center taps via Wc13^-1 ----
    G = feats64[rep]
    Winv = np.linalg.inv(fo_w[13].astype(f64))
    k = 0
    for dx in (-1, 0, 1):
        for dy in (-1, 0, 1):
            for dz in (-1, 0, 1):
                if (dx, dy, dz) != (0, 0, 0):
                    nk = key + (dx * HASH_D + dy) * HASH_D + dz
                    p = np.clip(np.searchsorted(skey, nk), 0, N - 1)
                    hit = skey[p] == nk
                    if hit.any():
                        dst = np.nonzero(hit)[0]
                        src = order[p[hit]]
                        A = fo_w[k].astype(f64) @ Winv
                        np.add.at(G, dst, feats64[src] @ A)
                k += 1

    # ---- shared scalar columns ----
    mx = (coords_xyz.max(0) + 1).astype(f64) * VS
    mn = (coords_xyz.min(0) - 1).astype(f64) * VS
    bias65 = np.zeros(65, f64)
    bias65[0:N_CLS] = sem_b.astype(f64)
    bias65[32:35] = c3
    bias65[64] = -cen_w.astype(f64).sum(0)[0]

    sc = np.zeros((C, 11), np.float32)
    sc[:, 0] = b1
    sc[:, 1] = bc
    sc[:, 2] = b2
    sc[0:65, 3] = bias65
    sc[32:35, 4] = mn
    sc[32:35, 5] = mx
    sc[:, 6] = b1 + 1.0
    sc[:, 7] = bc + 1.0
    sc[:, 8] = b2 + 1.0

    nwb = 406 if zero_mask else 660
    wb = np.zeros((C, nwb), BF16)
    wb[:, 0:128] = W1.astype(BF16)
    wb[:, 128:256] = Wc.astype(BF16)
    wb[:, 256:384] = W2.astype(BF16)
    if zero_mask:
        wb[:, 384:402] = sem_w.astype(f64).astype(BF16)
        wb[:, 402:405] = W3.astype(BF16)
        wb[:, 405:406] = cen_w.astype(f64).astype(BF16)
    else:
        # j-major layout: rows [cls 0:18 | cen 18 | regJ 19:127]
        sc64 = scales.astype(f64)
        semJF = np.zeros((C, FOH), f64)
        fohF = np.zeros((C, FOH), f64)
        thrF = np.zeros(FOH, np.float32)
        biasF = np.zeros(FOH, np.float32)
        semJF[:, 0:N_CLS] = sem_w.astype(f64)
        fohF[:, 0:N_CLS] = cls_w.astype(f64)
        thrF[0:N_CLS] = LOGIT_THR - sem_b.astype(f64)
        biasF[0:N_CLS] = cls_b.astype(f64) - cls_w.astype(f64).sum(0)
        fohF[:, N_CLS] = cen_w.astype(f64)[:, 0]
        thrF[N_CLS] = -1e30
        biasF[N_CLS] = -cen_w.astype(f64).sum(0)[0]
        for j in range(N_REG):
            for cc in range(N_CLS):
                r = 19 + j * N_CLS + cc
                semJF[:, r] = sem_w.astype(f64)[:, cc]
                fohF[:, r] = reg_w.astype(f64)[:, j] * sc64[cc]
                thrF[r] = LOGIT_THR - sem_b.astype(f64)[cc]
                biasF[r] = -reg_w.astype(f64)[:, j].sum() * sc64[cc]
        wb[:, 384:511] = semJF.astype(BF16)
        wb[:, 511:638] = fohF.astype(BF16)
        wb[:, 638:641] = W3.astype(BF16)
        wb[:, 641:659] = sem_w.astype(f64).astype(BF16)
        wb[:, 659:660] = cen_w.astype(f64).astype(BF16)
        sc[0:FOH, 9] = thrF
        sc[0:FOH, 10] = biasF

    # ---- transposed, channel-major activations ----
    fT = np.ascontiguousarray(feats.T)
    gTf = np.ascontiguousarray(G.astype(np.float32).T)
    cT = coords_xyz.T.astype(np.float32) * VS

    wts = {"wb": wb, "sc": sc}
    in_maps = []
    for c in range(N_CORES):
        s, e = c * PER_CORE, (c + 1) * PER_CORE
        m = dict(wts)
        m["xT"] = np.ascontiguousarray(fT[:, s:e]).astype(BF16)
        m["gT"] = np.ascontiguousarray(gTf[:, s:e]).astype(BF16)
        m["cvs"] = np.ascontiguousarray(cT[:, s:e]).astype(BF16)
        in_maps.append(m)
    return in_maps, zero_mask


_CACHED = {}

# full path: regJ row 19 + j*18 + c  ->  output col 43 + c*6 + j
_REG_COLS = np.empty(FOH - 19, np.int64)
for _j in range(N_REG):
    for _c in range(N_CLS):
        _REG_COLS[_j * N_CLS + _c] = 43 + _c * N_REG + _j


def _assemble(res, zero_mask, n):
    """Map device outputs to reference layout [n, 151] f32."""
    o = np.zeros((n, OUT_ROWS), np.float32)
    outH = res["outH"][:, :n].astype(np.float32)
    o[:, 0:18] = outH[0:18].T
    o[:, 18:21] = outH[32:35].T
    o[:, 21:24] = res["outW"][:, :n].astype(np.float32).T
    o[:, 24] = outH[64]
    if not zero_mask:
        outB = res["outB"][:, :n].astype(np.float32)
        o[:, 25:43] = outB[0:N_CLS].T
        o[:, _REG_COLS] = outB[19:FOH].T
    return o


def kernel(**inputs):
    inputs = {k: np.asarray(v) for k, v in inputs.items()}
    in_maps, zero_mask = _host_prep(**inputs)
    ckey = "fast" if zero_mask else "full"
    if ckey not in _CACHED:
        _CACHED[ckey] = _build_fast() if zero_mask else _build_full()
    nc = _CACHED[ckey]
    res = run_bass_kernel_spmd(nc, in_maps, core_ids=list(range(N_CORES)))
    out = np.empty((N_VOX, OUT_ROWS), np.float32)
    for c in range(N_CORES):
        out[c * PER_CORE:(c + 1) * PER_CORE] = _assemble(
            res.results[c], zero_mask, PER_CORE)
    return out
